# revision 24
# baseline (speedup 1.0000x reference)
"""AllCostVolume Trainium2 kernel: 8-core SPMD, query-point sharded.

Stage 1 (per query point n, fused in SBUF):
  72-channel pairwise feature MLP1 -> attention over N2 -> pi_feat,
  computed via a K=195 contraction decomposition: the pe*qe / rm / cm
  feature channels are folded into per-n stationary weights (DVE-built)
  against static rhs matrices [qeT; bT], [bicT], [f2xT].
Stage 2: KNN top-16 via max8/match_replace, indirect-DMA gathers,
  small MLP over (n,k) pairs, softmax over k.
Cross-core: AllGather of pi_feat (128x64 per core).
"""
import numpy as np

from concourse import bass, bacc, mybir
from concourse.tile import TileContext
from concourse.bass_utils import run_bass_kernel_spmd

f32 = mybir.dt.float32
bf16 = mybir.dt.bfloat16
u32 = mybir.dt.uint32
AF = mybir.ActivationFunctionType
OP = mybir.AluOpType

NC_ = 8          # cores
N = 1024         # query points
M = N // NC_     # per-core query points = 128
N2 = 1024
C = 64
KNN = 16
BN = 1.0 / np.sqrt(1.0 + 1e-5)

LAST_EXEC_NS = None
LAST_RESULT = None



# (name, rows, cols) — packed into one (128 x NCOLS) constant block
CONST_LAYOUT = [
    ('rhsA', 128, 1024), ('bT', 64, 1024), ('f2xT', 3, 1024),
    ('aT', 64, 1024), ('bsb', 128, 512), ('d2rhs', 5, 1024),
    ('ident', 128, 128),
    ('W1mid', 64, 128), ('W1f2', 3, 128), ('W70bc', 64, 128),
    ('W71bc', 64, 128), ('W1xyz', 3, 128),
    ('W2dup', 128, 128), ('W3d2', 128, 128), ('W4', 128, 128),
    ('W4r', 128, 128), ('W5dup', 128, 128),
    ('Wp36', 3, 64), ('Wp03', 3, 64), ('Wpc', 10, 64),
    ('W6a', 128, 128), ('W6b', 64, 128), ('W7', 128, 64),
    ('ones3', 3, 1),
    ('s1col', 128, 1), ('cb1col', 128, 1), ('s2dup', 128, 1),
    ('cb2dup', 128, 1), ('s3col', 64, 1), ('cb3col', 64, 1),
    ('s4col', 128, 1), ('cb4col', 128, 1), ('s5dup', 128, 1),
    ('cb5dup', 128, 1), ('spcol', 64, 1), ('cbpcol', 64, 1),
    ('spccol', 64, 1), ('cbpccol', 64, 1), ('s6col', 128, 1),
    ('cb6col', 128, 1), ('s7col', 64, 1), ('cb7col', 64, 1),
    ('aT_m', 64, 128), ('a_m', 128, 64), ('peT_m', 64, 128),
    ('wxyzT_m', 3, 128), ('wpT_m', 64, 128), ('d2lhsT', 5, 128),
]
BF16_LAYOUT = [
    ('qeT_b', 64, 1024), ('rhsB2_b', 67, 1024),
    ('W1mid_b', 64, 128), ('W1f2_b', 3, 128), ('W70bc_b', 64, 128),
    ('W71bc_b', 64, 128),
    ('W2dup_b', 128, 128), ('W3d2_b', 128, 128), ('W4_b', 128, 128),
    ('W4r_b', 128, 128), ('W5dup_b', 128, 128),
    ('Wpc_b', 10, 64), ('W6a_b', 128, 128), ('W6b_b', 64, 128),
    ('W7_b', 128, 64),
    ('ones512row_b', 1, 512), ('cb3duprow_b', 1, 128), ('cb4row_b', 1, 128),
    ('cb2duprow_b', 1, 128),
]
BF16_COLS = sum(c for _, _, c in BF16_LAYOUT)
BF16_OFF = {}
_o2 = 0
for _n, _r, _c in BF16_LAYOUT:
    BF16_OFF[_n] = _o2
    _o2 += _c

CONST_COLS = sum(c for _, _, c in CONST_LAYOUT)
CONST_OFF = {}
_o = 0
for _n, _r, _c in CONST_LAYOUT:
    CONST_OFF[_n] = _o
    _o += _c


def _fold(p):
    W, b, g, be = [np.asarray(x, np.float32) for x in p]
    s = (g * BN).astype(np.float32)
    cb = (b * s + be).astype(np.float32)
    return W, s, cb


def _col(x):
    return np.ascontiguousarray(np.asarray(x, np.float32).reshape(-1, 1))


def _host_prep(inputs):
    wxyz_in = np.asarray(inputs['warped_xyz'][0], np.float32)
    wp = np.asarray(inputs['warped_points'][0], np.float32)
    f2x = np.asarray(inputs['f2_xyz'][0], np.float32)
    f2p = np.asarray(inputs['f2_points'][0], np.float32)
    lz = np.asarray(inputs['lidar_z'][0], np.float32)

    W1, s1, cb1 = _fold(inputs['mlp1_params'][0])
    W2, s2, cb2 = _fold(inputs['mlp1_params'][1])
    W3, s3, cb3 = _fold(inputs['mlp1_params'][2])
    W4, s4, cb4 = _fold(inputs['mlp2_params'][0])
    W5, s5, cb5 = _fold(inputs['mlp2_params'][1])
    Wp, sp, cbp = _fold(inputs['pi_enc'])
    Wpc, spc, cbpc = _fold(inputs['pc_enc'])
    W6, s6, cb6 = _fold(inputs['mlp2b_params'][0])
    W7, s7, cb7 = _fold(inputs['mlp2b_params'][1])

    wxyz = (wxyz_in * lz).astype(np.float32)                     # (1024,3)
    a = wp / np.linalg.norm(wp, axis=1, keepdims=True)
    b = f2p / np.linalg.norm(f2p, axis=1, keepdims=True)
    pe = (wp - wp.mean(1, keepdims=True)) / wp.std(1, keepdims=True, ddof=1)
    qe = (f2p - f2p.mean(1, keepdims=True)) / f2p.std(1, keepdims=True, ddof=1)
    sq = (wxyz * wxyz).sum(1).astype(np.float32)

    cc = np.ascontiguousarray
    W2dup = cc(np.concatenate([W2, W2], 1))                      # (128,128)
    W3d2 = np.zeros((128, 128), np.float32)
    W3d2[0:64, 0:64] = W3
    W3d2[64:128, 64:128] = W3
    W5dup = cc(np.concatenate([W5, W5], 1))                      # (128,128)

    rep = {
        'rhsA': cc(np.concatenate([qe.T, b.T], 0)),
        'bT': cc(b.T),              # (128,1024)
        'f2xT': cc(f2x.T),                                       # (3,1024)
        'aT': cc(a.T),                                           # (64,1024)
        'bsb': cc(b.reshape(8, 128, 64).transpose(1, 0, 2).reshape(128, 512)),
        'd2rhs': cc(np.concatenate([wxyz.T, np.ones((1, N), np.float32),
                                    sq[None, :]], 0)),           # (5,1024)
        'wxyz4': cc(np.concatenate([wxyz, np.zeros((N, 1), np.float32)], 1)),
        'ident': np.eye(128, dtype=np.float32),
        'W1mid': cc(W1[6:70]), 'W1f2': cc(W1[3:6]),
        'W70bc': cc(np.repeat(W1[70][None, :], 64, 0)),
        'W71bc': cc(np.repeat(W1[71][None, :], 64, 0)),
        'W1xyz': cc(W1[0:3]),
        's1col': _col(s1), 'cb1col': _col(cb1),
        'W2dup': W2dup, 's2dup': _col(np.concatenate([s2, s2])),
        'cb2dup': _col(np.concatenate([cb2, cb2])),
        'W3d2': W3d2, 's3col': _col(s3), 'cb3col': _col(cb3),
        'W4': cc(W4), 's4col': _col(s4), 'cb4col': _col(cb4),
        'W4r': cc(np.concatenate([W4[64:128], W4[0:64]], 0)),
        'W5dup': W5dup, 's5dup': _col(np.concatenate([s5, s5])),
        'cb5dup': _col(np.concatenate([cb5, cb5])),
        'Wp36': cc(Wp[3:6]), 'Wp03': cc(Wp[0:3]),
        'spcol': _col(sp), 'cbpcol': _col(cbp),
        'Wpc': cc(Wpc), 'spccol': _col(spc), 'cbpccol': _col(cbpc),
        'W6a': cc(W6[0:128]), 'W6b': cc(W6[128:192]),
        's6col': _col(s6), 'cb6col': _col(cb6),
        'W7': cc(W7), 's7col': _col(s7), 'cb7col': _col(cb7),
        'ones3': np.ones((3, 1), np.float32),
    }
    in_maps = []
    for c in range(NC_):
        s = slice(c * M, (c + 1) * M)
        d = dict(rep)
        d['aT_m'] = cc(a[s].T)                                   # (64,128)
        d['a_m'] = cc(a[s])                                      # (128,64)
        d['peT_m'] = cc(pe[s].T)                                 # (64,128)
        d['wxyzT_m'] = cc(wxyz[s].T)                             # (3,128)
        d['wpT_m'] = cc(wp[s].T)                                 # (64,128)
        d['d2lhsT'] = cc(np.concatenate(
            [-2.0 * wxyz[s].T, sq[s][None, :],
             np.ones((1, M), np.float32)], 0))                   # (5,128)
        blob = np.zeros((128, CONST_COLS), np.float32)
        for nm, r, ccols in CONST_LAYOUT:
            t = d[nm]
            assert t.shape == (r, ccols), (nm, t.shape, (r, ccols))
            blob[0:r, CONST_OFF[nm]:CONST_OFF[nm] + ccols] = t
        import ml_dtypes
        bset = {
            'qeT_b': qe.T, 'rhsB2_b': np.concatenate([b.T, f2x.T], 0),
            'W1mid_b': W1[6:70], 'W1f2_b': W1[3:6],
            'W70bc_b': np.repeat(W1[70][None, :], 64, 0),
            'W71bc_b': np.repeat(W1[71][None, :], 64, 0),
            'W2dup_b': W2dup, 'W3d2_b': W3d2, 'W4_b': W4,
            'W4r_b': np.concatenate([W4[64:128], W4[0:64]], 0),
            'W5dup_b': W5dup,
            'Wpc_b': Wpc, 'W6a_b': W6[0:128], 'W6b_b': W6[128:192],
            'W7_b': W7,
            'ones512row_b': np.ones((1, 512), np.float32),
            'cb3duprow_b': np.concatenate([cb3, cb3])[None, :],
            'cb4row_b': cb4[None, :],
            'cb2duprow_b': np.concatenate([cb2, cb2])[None, :],
        }
        blob2 = np.zeros((128, BF16_COLS), ml_dtypes.bfloat16)
        for nm, r, ccols in BF16_LAYOUT:
            t = np.asarray(bset[nm], np.float32)
            assert t.shape == (r, ccols), (nm, t.shape, (r, ccols))
            blob2[0:r, BF16_OFF[nm]:BF16_OFF[nm] + ccols] = t.astype(
                ml_dtypes.bfloat16)
        in_maps.append({'CONST': blob, 'CONSTB': blob2,
                        'wxyz4': rep['wxyz4']})
    return in_maps


_SHAPES = None


def _build(timing_mode=False):
    nc = bacc.Bacc()
    P = {}
    P['CONST'] = nc.declare_dram_parameter('CONST', [128, CONST_COLS], f32,
                                           isOutput=False)
    P['CONSTB'] = nc.declare_dram_parameter('CONSTB', [128, BF16_COLS],
                                            bf16, isOutput=False)
    P['wxyz4'] = nc.declare_dram_parameter('wxyz4', [N, 4], f32,
                                           isOutput=False)
    out_p = nc.declare_dram_parameter("out", [M, 64], f32, isOutput=True)

    pf_mine = nc.dram_tensor("pf_mine", [M, 64], f32)
    pf_full = nc.dram_tensor("pf_full", [N, 64], f32, addr_space="Shared")

    with TileContext(nc) as tc:
        v = nc.vector
        sc = nc.scalar
        te = nc.tensor

        import contextlib
        est = contextlib.ExitStack()
        with est:
            const = est.enter_context(tc.tile_pool(name="const", bufs=1))

            cblk = const.tile([128, CONST_COLS], f32, tag="cblk")
            nc.sync.dma_start(out=cblk[:], in_=P['CONST'][:])

            def load(name):
                r = dict((n, rr) for n, rr, _ in CONST_LAYOUT)[name]
                c = dict((n, cc_) for n, _, cc_ in CONST_LAYOUT)[name]
                o = CONST_OFF[name]
                return cblk[0:r, o:o + c]

            cblk2 = const.tile([128, BF16_COLS], bf16, tag="cblk2")
            nc.sync.dma_start(out=cblk2[:], in_=P['CONSTB'][:])

            def loadb(name):
                r = dict((n, rr) for n, rr, _ in BF16_LAYOUT)[name]
                c = dict((n, cc_) for n, _, cc_ in BF16_LAYOUT)[name]
                o = BF16_OFF[name]
                return cblk2[0:r, o:o + c]

            # ---- static SBUF loads ----
            rhsA = load('rhsA')
            bT = load('bT')
            f2xT = load('f2xT')
            aT = load('aT')
            bsb = load('bsb')
            d2rhs = load('d2rhs')
            ident = load('ident')
            W1mid = loadb('W1mid_b'); W1f2 = loadb('W1f2_b')
            W70bc = loadb('W70bc_b'); W71bc = loadb('W71bc_b')
            W1xyz = load('W1xyz')
            qeTb = loadb('qeT_b'); rhsB2 = loadb('rhsB2_b')
            ones512row = loadb('ones512row_b')
            cb3duprow = loadb('cb3duprow_b'); cb4row = loadb('cb4row_b')
            cb2duprow = loadb('cb2duprow_b')
            s1col = load('s1col'); cb1col = load('cb1col')
            W2dup = loadb('W2dup_b'); s2dup = load('s2dup'); cb2dup = load('cb2dup')
            W3d2 = loadb('W3d2_b'); s3col = load('s3col'); cb3col = load('cb3col')
            W4 = loadb('W4_b'); W4r = loadb('W4r_b')
            s4col = load('s4col'); cb4col = load('cb4col')
            W5dup = loadb('W5dup_b'); s5dup = load('s5dup'); cb5dup = load('cb5dup')
            Wp36 = load('Wp36'); Wp03 = load('Wp03')
            spcol = load('spcol'); cbpcol = load('cbpcol')
            Wpc = loadb('Wpc_b'); spccol = load('spccol'); cbpccol = load('cbpccol')
            W6a = loadb('W6a_b'); W6b = loadb('W6b_b')
            s6col = load('s6col'); cb6col = load('cb6col')
            W7 = loadb('W7_b'); s7col = load('s7col'); cb7col = load('cb7col')
            ones3 = load('ones3')
            aT_m = load('aT_m'); a_m = load('a_m'); peT_m = load('peT_m')
            wxyzT_m = load('wxyzT_m'); wpT_m = load('wpT_m')
            d2lhsT = load('d2lhsT')

            rhsA2 = const.tile([128, 1024], bf16, tag="rhsA2")
            SVpT = const.tile([64, 1024], bf16, tag="SVpT")
            bias1T = const.tile([128, 128], f32, tag="bias1T")
            biaspT = const.tile([64, 128], f32, tag="biaspT")
            airT = const.tile([64, 128], f32, tag="airT")
            numT = const.tile([128, 64], f32, tag="numT")
            denT = const.tile([128, 64], f32, tag="denT")
            idxs = const.tile([128, 16], u32, tag="idxs")
            nd2 = const.tile([128, 1024], f32, tag="nd2")
            nd2b = const.tile([128, 1024], f32, tag="nd2b")
            gfT = const.tile([64, 2048], bf16, tag="gfT")
            pcrhs = const.tile([10, 2048], bf16, tag="pcrhs")
            ones128 = const.tile([128, 1024], bf16, tag="ones128")
            v.memset(ones128[:], 1.0)
            v.tensor_copy(out=rhsA2[0:64, :], in_=qeTb[:])

            # =========== prep ===========
            with tc.tile_pool(name="ppp", bufs=2, space="PSUM") as ppp, \
                 tc.tile_pool(name="pps", bufs=2) as pps:
                # bias1T = s1*(W1xyz^T @ wxyzT_m) + cb1
                u1p = ppp.tile([128, 128], f32, tag="pp")
                te.matmul(out=u1p[:], lhsT=W1xyz[:], rhs=wxyzT_m[:],
                          start=True, stop=True)
                v.tensor_scalar(out=bias1T[:], in0=u1p[:], scalar1=s1col[:],
                                scalar2=cb1col[:], op0=OP.mult, op1=OP.add)
                # biaspT = sp*(Wp03^T @ wxyzT_m) + cbp
                upp = ppp.tile([64, 128], f32, tag="pp")
                te.matmul(out=upp[:], lhsT=Wp03[:], rhs=wxyzT_m[:],
                          start=True, stop=True)
                v.tensor_scalar(out=biaspT[:], in0=upp[:], scalar1=spcol[:],
                                scalar2=cbpcol[:], op0=OP.mult, op1=OP.add)
                # SVpT = sp * (Wp36^T @ f2xT)
                for cch in range(2):
                    svp = ppp.tile([64, 512], f32, tag="pp")
                    te.matmul(out=svp[:], lhsT=Wp36[:],
                              rhs=f2xT[:, cch * 512:(cch + 1) * 512],
                              start=True, stop=True)
                    sc.activation(out=SVpT[:, cch * 512:(cch + 1) * 512],
                                  in_=svp[:], func=AF.Copy, scale=spcol[:])
                # rowmax of cos_mine -> ir -> airT
                rmx = pps.tile([128, 2], f32, tag="rmx")
                for cch in range(2):
                    cmp_ = ppp.tile([128, 512], f32, tag="pp")
                    te.matmul(out=cmp_[:], lhsT=aT_m[:],
                              rhs=bT[:, cch * 512:(cch + 1) * 512],
                              start=True, stop=True)
                    v.tensor_reduce(out=rmx[:, cch:cch + 1], in_=cmp_[:],
                                    axis=mybir.AxisListType.X, op=OP.max)
                irc = pps.tile([128, 1], f32, tag="irc")
                v.tensor_tensor(out=irc[:], in0=rmx[:, 0:1], in1=rmx[:, 1:2],
                                op=OP.max)
                v.tensor_scalar(out=irc[:], in0=irc[:], scalar1=1e-10,
                                scalar2=None, op0=OP.add)
                v.reciprocal(out=irc[:], in_=irc[:])
                air = pps.tile([128, 64], f32, tag="air")
                v.tensor_scalar(out=air[:], in0=a_m[:], scalar1=irc[:],
                                scalar2=None, op0=OP.mult)
                airp = ppp.tile([64, 128], f32, tag="pp")
                te.transpose(out=airp[:], in_=air[:], identity=ident[:])
                v.tensor_copy(out=airT[:], in_=airp[:])
                # colmax over all n per m-tile -> ic -> bic -> bicT (rhsB64)
                for t in range(8):
                    cmx = pps.tile([128, 2], f32, tag="cmx")
                    for cch in range(2):
                        ctp = ppp.tile([128, 512], f32, tag="pp")
                        te.matmul(out=ctp[:],
                                  lhsT=bT[:, t * 128:(t + 1) * 128],
                                  rhs=aT[:, cch * 512:(cch + 1) * 512],
                                  start=True, stop=True)
                        v.tensor_reduce(out=cmx[:, cch:cch + 1], in_=ctp[:],
                                        axis=mybir.AxisListType.X, op=OP.max)
                    icc = pps.tile([128, 1], f32, tag="icc")
                    v.tensor_tensor(out=icc[:], in0=cmx[:, 0:1],
                                    in1=cmx[:, 1:2], op=OP.max)
                    v.tensor_scalar(out=icc[:], in0=icc[:], scalar1=1e-10,
                                    scalar2=None, op0=OP.add)
                    v.reciprocal(out=icc[:], in_=icc[:])
                    bict = pps.tile([128, 64], f32, tag="bict")
                    v.tensor_scalar(out=bict[:],
                                    in0=bsb[:, t * 64:(t + 1) * 64],
                                    scalar1=icc[:], scalar2=None, op0=OP.mult)
                    bicp = ppp.tile([64, 128], f32, tag="pp")
                    te.transpose(out=bicp[:], in_=bict[:], identity=ident[:])
                    v.tensor_copy(out=rhsA2[64:128, t * 128:(t + 1) * 128],
                                  in_=bicp[:])
                # d2 -> nd2 -> top-16 indices
                d2p = ppp.tile([128, 1024], f32, tag="d2p")
                for cch in range(2):
                    te.matmul(out=d2p[:, cch * 512:(cch + 1) * 512],
                              lhsT=d2lhsT[:],
                              rhs=d2rhs[:, cch * 512:(cch + 1) * 512],
                              start=True, stop=True)
                sc.activation(out=nd2[:], in_=d2p[:], func=AF.Copy, scale=-1.0)
                mx8a = pps.tile([128, 8], f32, tag="mx8")
                v.max(out=mx8a[:], in_=nd2[:])
                v.max_index(out=idxs[:, 0:8], in_max=mx8a[:], in_values=nd2[:])
                v.match_replace(out=nd2b[:], in_to_replace=mx8a[:],
                                in_values=nd2[:], imm_value=-3.0e38)
                mx8b = pps.tile([128, 8], f32, tag="mx8")
                v.max(out=mx8b[:], in_=nd2b[:])
                v.max_index(out=idxs[:, 8:16], in_max=mx8b[:], in_values=nd2b[:])

            # =========== stage-1 loop over pairs of query points ===========
            with tc.tile_pool(name="psA", bufs=2, space="PSUM") as psA, \
                 tc.tile_pool(name="psB", bufs=3, space="PSUM") as psB, \
                 tc.tile_pool(name="lsb", bufs=3) as lsb:
                for i in range(M // 2):
                    nn = [2 * i, 2 * i + 1]
                    y1s = []
                    cats = []
                    h1s = []
                    for half, n in enumerate(nn):
                        # lA2: [pe*W1mid ; a*W71] vs rhsA2=[qeT; bicT]
                        lA = lsb.tile([128, 128], bf16, tag="lA")
                        v.tensor_scalar(out=lA[0:64, :], in0=W1mid[:],
                                        scalar1=peT_m[:, n:n + 1],
                                        scalar2=None, op0=OP.mult)
                        v.tensor_scalar(out=lA[64:128, :], in0=W71bc[:],
                                        scalar1=aT_m[:, n:n + 1],
                                        scalar2=None, op0=OP.mult)
                        # lB2: [ir*a*W70 ; W1f2] vs rhsB2=[bT; f2xT]
                        lB = lsb.tile([67, 128], bf16, tag="lB")
                        v.tensor_scalar(out=lB[0:64, :], in0=W70bc[:],
                                        scalar1=airT[:, n:n + 1],
                                        scalar2=None, op0=OP.mult)
                        v.tensor_copy(out=lB[64:67, :], in_=W1f2[:])
                        y1 = lsb.tile([128, 1024], bf16, tag=f"y1s{half}")
                        for cch in range(2):
                            cs = slice(cch * 512, (cch + 1) * 512)
                            y1p = psA.tile([128, 512], f32, tag="y1")
                            te.matmul(out=y1p[:], lhsT=lA[:],
                                      rhs=rhsA2[:, cs], start=True, stop=False)
                            te.matmul(out=y1p[:], lhsT=lB[:],
                                      rhs=rhsB2[:, cs], start=False, stop=True)
                            sc.activation(out=y1[:, cs], in_=y1p[:],
                                          func=AF.Relu,
                                          bias=bias1T[:, n:n + 1],
                                          scale=s1col[:])
                        y1s.append(y1)
                        cat = lsb.tile([128, 1024], bf16, tag=f"cat{half}")
                        pi_rows = slice(0, 64) if half == 0 else slice(64, 128)
                        v.tensor_scalar(
                            out=cat[pi_rows, :], in0=SVpT[:],
                            scalar1=biaspT[:, n:n + 1], scalar2=0.0,
                            op0=OP.add, op1=OP.max)
                        cats.append(cat)
                    # y2 (both points packed on partitions)
                    y2s = lsb.tile([128, 1024], bf16, tag="y2s")
                    for cch in range(2):
                        cs = slice(cch * 512, (cch + 1) * 512)
                        y2p = psB.tile([128, 512], f32, tag="midA")
                        te.matmul(out=y2p[0:64, :], lhsT=W2dup[:, 0:64],
                                  rhs=y1s[0][:, cs], start=True, stop=True)
                        te.matmul(out=y2p[64:128, :], lhsT=W2dup[:, 64:128],
                                  rhs=y1s[1][:, cs], start=True, stop=True)
                        sc.activation(out=y2s[:, cs], in_=y2p[:], func=AF.Relu,
                                      bias=cb2dup[:], scale=s2dup[:])
                    # y3
                    for cch in range(2):
                        cs = slice(cch * 512, (cch + 1) * 512)
                        y3p = psB.tile([128, 512], f32, tag="midA")
                        te.matmul(out=y3p[:], lhsT=cb3duprow[:],
                                  rhs=ones512row[:], start=True, stop=False)
                        te.matmul(out=y3p[0:64, :], lhsT=W3d2[0:64, 0:64],
                                  rhs=y2s[0:64, cs], start=False, stop=True)
                        te.matmul(out=y3p[64:128, :], lhsT=W3d2[64:128, 64:128],
                                  rhs=y2s[64:128, cs], start=False, stop=True,
                                  tile_position=(64, 64))
                        sc.activation(out=cats[0][64:128, cs], in_=y3p[0:64, :],
                                      func=AF.Relu, scale=s3col[:])
                        v.tensor_scalar(out=cats[1][0:64, cs],
                                        in0=y3p[64:128, :],
                                        scalar1=s3col[:], scalar2=0.0,
                                        op0=OP.mult, op1=OP.max)
                    # h1 per point
                    for half, n in enumerate(nn):
                        w4t = W4 if half == 0 else W4r
                        h1 = lsb.tile([128, 1024], bf16, tag=f"h1s{half}")
                        for cch in range(2):
                            cs = slice(cch * 512, (cch + 1) * 512)
                            h1p = psB.tile([128, 512], f32, tag="midB")
                            te.matmul(out=h1p[:], lhsT=cb4row[:],
                                      rhs=ones512row[:], start=True, stop=False)
                            te.matmul(out=h1p[:], lhsT=w4t[:],
                                      rhs=cats[half][:, cs],
                                      start=False, stop=True)
                            v.tensor_scalar(out=h1[:, cs], in0=h1p[:],
                                            scalar1=s4col[:], scalar2=0.0,
                                            op0=OP.mult, op1=OP.max)
                        h1s.append(h1)
                    # h -> exp
                    texp = lsb.tile([128, 1024], bf16, tag="texp")
                    for cch in range(2):
                        cs = slice(cch * 512, (cch + 1) * 512)
                        hp = psB.tile([128, 512], f32, tag="midB")
                        te.matmul(out=hp[0:64, :], lhsT=W5dup[:, 0:64],
                                  rhs=h1s[1][:, cs], start=True, stop=True)
                        te.matmul(out=hp[64:128, :], lhsT=W5dup[:, 64:128],
                                  rhs=h1s[0][:, cs], start=True, stop=True)
                        sc.activation(out=texp[:, cs], in_=hp[:], func=AF.Exp,
                                      bias=cb5dup[:], scale=s5dup[:])
                    # num/den with fused relu: E = max(exp(z),1)
                    scr = lsb.tile([128, 1024], bf16, tag="scr")
                    v.scalar_tensor_tensor(
                        out=scr[64:128, :], in0=texp[64:128, :], scalar=1.0,
                        in1=cats[0][64:128, :], op0=OP.max, op1=OP.mult,
                        accum_out=numT[64:128, i:i + 1])
                    v.scalar_tensor_tensor(
                        out=scr[0:64, :], in0=texp[0:64, :], scalar=1.0,
                        in1=cats[1][0:64, :], op0=OP.max, op1=OP.mult,
                        accum_out=numT[0:64, i:i + 1])
                    scr2 = lsb.tile([128, 1024], bf16, tag="scr2")
                    v.scalar_tensor_tensor(
                        out=scr2[:], in0=texp[:], scalar=1.0,
                        in1=ones128[:], op0=OP.max, op1=OP.mult,
                        accum_out=denT[:, i:i + 1])

            # =========== pi_feat assembly + AllGather ===========
            with tc.tile_pool(name="ps2", bufs=2, space="PSUM") as ps2, \
                 tc.tile_pool(name="sb2", bufs=1) as sb2, \
                 tc.tile_pool(name="gsb", bufs=3) as gsb:
                denR = sb2.tile([128, 64], f32, tag="denR")
                v.reciprocal(out=denR[:], in_=denT[:])
                pfZ = sb2.tile([128, 64], f32, tag="pfZ")
                v.tensor_tensor(out=pfZ[:], in0=numT[:], in1=denR[:],
                                op=OP.mult)
                pfZ2 = sb2.tile([128, 64], f32, tag="pfZ2")
                v.tensor_copy(out=pfZ2[0:64, :], in_=pfZ[64:128, :])
                v.tensor_copy(out=pfZ2[64:128, :], in_=pfZ[0:64, :])
                pfp = ps2.tile([64, 128], f32, tag="tp")
                te.transpose(out=pfp[:], in_=pfZ2[:], identity=ident[:])
                pfs = sb2.tile([64, 128], f32, tag="pfs")
                v.tensor_copy(out=pfs[:], in_=pfp[:])
                nc.sync.dma_start(
                    out=pf_mine[:].rearrange("(i h) j -> i (h j)", h=2),
                    in_=pfs[:])
                if timing_mode:
                    nc.sync.dma_start(out=pf_full[0:M, :], in_=pf_mine[:])
                else:
                    nc.gpsimd.collective_compute(
                        "AllGather", OP.bypass,
                        replica_groups=[list(range(NC_))],
                        ins=[pf_mine[:]],
                        outs=[pf_full[:]],
                    )
                gxTt = sb2.tile([3, 2048], bf16, tag="gxTt")
                wbc = sb2.tile([3, 2048], bf16, tag="wbc")
                # gathers + transposes into channel-major
                for k in range(KNN):
                    gf = gsb.tile([128, 64], f32, tag="gf")
                    nc.gpsimd.indirect_dma_start(
                        out=gf[:], out_offset=None, in_=pf_full[:],
                        in_offset=bass.IndirectOffsetOnAxis(
                            ap=idxs[:, k:k + 1], axis=0))
                    gfp = ps2.tile([64, 128], f32, tag="tp")
                    te.transpose(out=gfp[:], in_=gf[:], identity=ident[:])
                    v.tensor_copy(out=gfT[:, k * 128:(k + 1) * 128], in_=gfp[:])
                    gx = gsb.tile([128, 4], f32, tag="gx")
                    nc.gpsimd.indirect_dma_start(
                        out=gx[:], out_offset=None, in_=P['wxyz4'][:],
                        in_offset=bass.IndirectOffsetOnAxis(
                            ap=idxs[:, k:k + 1], axis=0))
                    gxp = ps2.tile([4, 128], f32, tag="tp4")
                    te.transpose(out=gxp[:], in_=gx[:], identity=ident[:])
                    v.tensor_copy(out=gxTt[:, k * 128:(k + 1) * 128],
                                  in_=gxp[0:3, :])
                    v.tensor_copy(out=wbc[:, k * 128:(k + 1) * 128],
                                  in_=wxyzT_m[:])

                # pc10 channels: [wxyz, g_xyz, diff, euc]
                v.tensor_copy(out=pcrhs[0:3, :], in_=wbc[:])
                nc.sync.dma_start(out=pcrhs[3:6, :], in_=gxTt[:])
                dft = sb2.tile([3, 2048], bf16, tag="dft")
                v.tensor_tensor(out=dft[:], in0=gxTt[:], in1=wbc[:],
                                op=OP.subtract)
                nc.sync.dma_start(out=pcrhs[6:9, :], in_=dft[:])
                ones3b = sb2.tile([3, 1], bf16, tag="ones3b")
                v.memset(ones3b[:], 1.0)
                dsq = sb2.tile([3, 2048], bf16, tag="dsq")
                v.tensor_tensor(out=dsq[:], in0=dft[:], in1=dft[:],
                                op=OP.mult)
                euct = sb2.tile([1, 2048], bf16, tag="euct")
                for cch in range(4):
                    cs = slice(cch * 512, (cch + 1) * 512)
                    ecp = ps2.tile([1, 512], f32, tag="sm")
                    te.matmul(out=ecp[:], lhsT=ones3b[:], rhs=dsq[:, cs],
                              start=True, stop=True)
                    sc.activation(out=euct[:, cs], in_=ecp[:], func=AF.Sqrt)
                nc.sync.dma_start(out=pcrhs[9:10, :], in_=euct[:])
                # pc_enc
                rhs2a = sb2.tile([128, 2048], bf16, tag="rhs2a")
                for cch in range(4):
                    cs = slice(cch * 512, (cch + 1) * 512)
                    pcp = ps2.tile([64, 512], f32, tag="big")
                    te.matmul(out=pcp[:], lhsT=Wpc[:], rhs=pcrhs[:, cs],
                              start=True, stop=True)
                    sc.activation(out=rhs2a[0:64, cs], in_=pcp[:],
                                  func=AF.Relu, bias=cbpccol[:],
                                  scale=spccol[:])
                for k in range(KNN):
                    v.tensor_copy(out=rhs2a[64:128, k * 128:(k + 1) * 128],
                                  in_=wpT_m[:])
                # mlp2b layer 1
                h1bs = sb2.tile([128, 2048], bf16, tag="h1bs")
                for cch in range(4):
                    cs = slice(cch * 512, (cch + 1) * 512)
                    h1bp = ps2.tile([128, 512], f32, tag="big")
                    te.matmul(out=h1bp[:], lhsT=W6a[:], rhs=rhs2a[:, cs],
                              start=True, stop=False)
                    te.matmul(out=h1bp[:], lhsT=W6b[:], rhs=gfT[:, cs],
                              start=False, stop=True)
                    sc.activation(out=h1bs[:, cs], in_=h1bp[:], func=AF.Relu,
                                  bias=cb6col[:], scale=s6col[:])
                # mlp2b layer 2 -> exp
                texp2 = sb2.tile([64, 2048], bf16, tag="texp2")
                for cch in range(4):
                    cs = slice(cch * 512, (cch + 1) * 512)
                    z2p = ps2.tile([64, 512], f32, tag="big")
                    te.matmul(out=z2p[:], lhsT=W7[:], rhs=h1bs[:, cs],
                              start=True, stop=True)
                    sc.activation(out=texp2[:, cs], in_=z2p[:], func=AF.Exp,
                                  bias=cb7col[:], scale=s7col[:])
                # num2 / den2 with segment reduce over k (free stride 128)
                scr2a = sb2.tile([64, 2048], bf16, tag="scr2a")
                v.scalar_tensor_tensor(out=scr2a[:], in0=texp2[:], scalar=1.0,
                                       in1=gfT[:], op0=OP.max, op1=OP.mult)
                num2 = sb2.tile([64, 128], f32, tag="num2")
                v.tensor_reduce(
                    out=num2[:],
                    in_=scr2a[:].rearrange("p (k n) -> p n k", k=KNN),
                    axis=mybir.AxisListType.X, op=OP.add)
                scr2b = sb2.tile([64, 2048], bf16, tag="scr2b")
                v.tensor_scalar(out=scr2b[:], in0=texp2[:], scalar1=1.0,
                                scalar2=None, op0=OP.max)
                den2 = sb2.tile([64, 128], f32, tag="den2")
                v.tensor_reduce(
                    out=den2[:],
                    in_=scr2b[:].rearrange("p (k n) -> p n k", k=KNN),
                    axis=mybir.AxisListType.X, op=OP.add)
                den2r = sb2.tile([64, 128], f32, tag="den2r")
                v.reciprocal(out=den2r[:], in_=den2[:])
                outT = sb2.tile([64, 128], f32, tag="outT")
                v.tensor_tensor(out=outT[:], in0=num2[:], in1=den2r[:],
                                op=OP.mult)
                outp = ps2.tile([128, 64], f32, tag="tp")
                te.transpose(out=outp[:], in_=outT[:],
                             identity=ident[0:64, 0:64])
                outs = sb2.tile([128, 64], f32, tag="outs")
                v.tensor_copy(out=outs[:], in_=outp[:])
                nc.sync.dma_start(out=out_p[:], in_=outs[:])

    nc.finalize()
    return nc


_NC_CACHE = None


def kernel(**inputs):
    global LAST_EXEC_NS, LAST_RESULT, _NC_CACHE
    in_maps = _host_prep(inputs)
    if _NC_CACHE is None:
        _NC_CACHE = _build()
    import os
    res = run_bass_kernel_spmd(
        _NC_CACHE, in_maps, core_ids=list(range(NC_)),
        trace=bool(os.environ.get("KERNEL_TRACE")),
    )
    LAST_RESULT = res
    LAST_EXEC_NS = res.exec_time_ns
    out = np.concatenate([res.results[i]["out"] for i in range(NC_)], 0)
    return out[None].astype(np.float32)


# revision 31
# speedup vs baseline: 1.1232x; 1.1232x over previous
"""AllCostVolume Trainium2 kernel: 8-core SPMD, query-point sharded.

Stage 1 (per query point n, fused in SBUF):
  72-channel pairwise feature MLP1 -> attention over N2 -> pi_feat,
  computed via a K=195 contraction decomposition: the pe*qe / rm / cm
  feature channels are folded into per-n stationary weights (DVE-built)
  against static rhs matrices [qeT; bT], [bicT], [f2xT].
Stage 2: KNN top-16 via max8/match_replace, indirect-DMA gathers,
  small MLP over (n,k) pairs, softmax over k.
Cross-core: AllGather of pi_feat (128x64 per core).
"""
import numpy as np

from concourse import bass, bacc, mybir
from concourse.tile import TileContext
from concourse.bass_utils import run_bass_kernel_spmd

f32 = mybir.dt.float32
bf16 = mybir.dt.bfloat16
u32 = mybir.dt.uint32
AF = mybir.ActivationFunctionType
OP = mybir.AluOpType

NC_ = 8          # cores
N = 1024         # query points
M = N // NC_     # per-core query points = 128
N2 = 1024
C = 64
KNN = 16
BN = 1.0 / np.sqrt(1.0 + 1e-5)

LAST_EXEC_NS = None
LAST_RESULT = None



# (name, rows, cols) — packed into one (128 x NCOLS) constant block
CONST_LAYOUT = [
    ('rhsA', 128, 1024), ('bT', 64, 1024), ('f2xT', 3, 1024),
    ('aT', 64, 1024), ('bsb', 128, 512), ('d2rhs', 5, 1024),
    ('ident', 128, 128),
    ('W1mid', 64, 128), ('W1f2', 3, 128), ('W70bc', 64, 128),
    ('W71bc', 64, 128), ('W1xyz', 3, 128),
    ('W2dup', 128, 128), ('W3d2', 128, 128), ('W4', 128, 128),
    ('W4r', 128, 128), ('W5dup', 128, 128),
    ('Wp36', 3, 64), ('Wp03', 3, 64), ('Wpc', 10, 64),
    ('W6a', 128, 128), ('W6b', 64, 128), ('W7', 128, 64),
    ('ones3', 3, 1),
    ('s1col', 128, 1), ('cb1col', 128, 1), ('s2dup', 128, 1),
    ('cb2dup', 128, 1), ('s3col', 64, 1), ('cb3col', 64, 1),
    ('s4col', 128, 1), ('cb4col', 128, 1), ('s5dup', 128, 1),
    ('cb5dup', 128, 1), ('spcol', 64, 1), ('cbpcol', 64, 1),
    ('spccol', 64, 1), ('cbpccol', 64, 1), ('s6col', 128, 1),
    ('cb6col', 128, 1), ('s7col', 64, 1), ('cb7col', 64, 1),
    ('aT_m', 64, 128), ('a_m', 128, 64), ('peT_m', 64, 128),
    ('wxyzT_m', 3, 128), ('wpT_m', 64, 128), ('d2lhsT', 5, 128),
]
BF16_LAYOUT = [
    ('qeT_b', 64, 1024), ('rhsB2_b', 67, 1024),
    ('W1mid_b', 64, 128), ('W1f2_b', 3, 128), ('W70bc_b', 64, 128),
    ('W71bc_b', 64, 128),
    ('W2dup_b', 128, 128), ('W3d2_b', 128, 128), ('W4_b', 128, 128),
    ('W4r_b', 128, 128), ('W5dup_b', 128, 128),
    ('Wpc_b', 10, 64), ('W6a_b', 128, 128), ('W6b_b', 64, 128),
    ('W7_b', 128, 64),
    ('ones512row_b', 1, 512), ('cb3duprow_b', 1, 128), ('cb4row_b', 1, 128),
    ('cb2duprow_b', 1, 128),
]
BF16_COLS = sum(c for _, _, c in BF16_LAYOUT)
BF16_OFF = {}
_o2 = 0
for _n, _r, _c in BF16_LAYOUT:
    BF16_OFF[_n] = _o2
    _o2 += _c

CONST_COLS = sum(c for _, _, c in CONST_LAYOUT)
CONST_OFF = {}
_o = 0
for _n, _r, _c in CONST_LAYOUT:
    CONST_OFF[_n] = _o
    _o += _c


def _fold(p):
    W, b, g, be = [np.asarray(x, np.float32) for x in p]
    s = (g * BN).astype(np.float32)
    cb = (b * s + be).astype(np.float32)
    return W, s, cb


def _col(x):
    return np.ascontiguousarray(np.asarray(x, np.float32).reshape(-1, 1))


def _host_prep(inputs):
    wxyz_in = np.asarray(inputs['warped_xyz'][0], np.float32)
    wp = np.asarray(inputs['warped_points'][0], np.float32)
    f2x = np.asarray(inputs['f2_xyz'][0], np.float32)
    f2p = np.asarray(inputs['f2_points'][0], np.float32)
    lz = np.asarray(inputs['lidar_z'][0], np.float32)

    W1, s1, cb1 = _fold(inputs['mlp1_params'][0])
    W2, s2, cb2 = _fold(inputs['mlp1_params'][1])
    W3, s3, cb3 = _fold(inputs['mlp1_params'][2])
    W4, s4, cb4 = _fold(inputs['mlp2_params'][0])
    W5, s5, cb5 = _fold(inputs['mlp2_params'][1])
    Wp, sp, cbp = _fold(inputs['pi_enc'])
    Wpc, spc, cbpc = _fold(inputs['pc_enc'])
    W6, s6, cb6 = _fold(inputs['mlp2b_params'][0])
    W7, s7, cb7 = _fold(inputs['mlp2b_params'][1])

    wxyz = (wxyz_in * lz).astype(np.float32)                     # (1024,3)
    a = wp / np.linalg.norm(wp, axis=1, keepdims=True)
    b = f2p / np.linalg.norm(f2p, axis=1, keepdims=True)
    pe = (wp - wp.mean(1, keepdims=True)) / wp.std(1, keepdims=True, ddof=1)
    qe = (f2p - f2p.mean(1, keepdims=True)) / f2p.std(1, keepdims=True, ddof=1)
    sq = (wxyz * wxyz).sum(1).astype(np.float32)

    cc = np.ascontiguousarray
    W2dup = cc(np.concatenate([W2, W2], 1))                      # (128,128)
    W3d2 = np.zeros((128, 128), np.float32)
    W3d2[0:64, 0:64] = W3
    W3d2[64:128, 64:128] = W3
    W5dup = cc(np.concatenate([W5, W5], 1))                      # (128,128)

    rep = {
        'rhsA': cc(np.concatenate([qe.T, b.T], 0)),
        'bT': cc(b.T),              # (128,1024)
        'f2xT': cc(f2x.T),                                       # (3,1024)
        'aT': cc(a.T),                                           # (64,1024)
        'bsb': cc(b.reshape(8, 128, 64).transpose(1, 0, 2).reshape(128, 512)),
        'd2rhs': cc(np.concatenate([wxyz.T, np.ones((1, N), np.float32),
                                    sq[None, :]], 0)),           # (5,1024)
        'wxyz4': cc(np.concatenate([wxyz, np.zeros((N, 1), np.float32)], 1)),
        'ident': np.eye(128, dtype=np.float32),
        'W1mid': cc(W1[6:70]), 'W1f2': cc(W1[3:6]),
        'W70bc': cc(np.repeat(W1[70][None, :], 64, 0)),
        'W71bc': cc(np.repeat(W1[71][None, :], 64, 0)),
        'W1xyz': cc(W1[0:3]),
        's1col': _col(s1), 'cb1col': _col(cb1),
        'W2dup': W2dup, 's2dup': _col(np.concatenate([s2, s2])),
        'cb2dup': _col(np.concatenate([cb2, cb2])),
        'W3d2': W3d2, 's3col': _col(s3), 'cb3col': _col(cb3),
        'W4': cc(W4), 's4col': _col(s4), 'cb4col': _col(cb4),
        'W4r': cc(np.concatenate([W4[64:128], W4[0:64]], 0)),
        'W5dup': W5dup, 's5dup': _col(np.concatenate([s5, s5])),
        'cb5dup': _col(np.concatenate([cb5, cb5])),
        'Wp36': cc(Wp[3:6]), 'Wp03': cc(Wp[0:3]),
        'spcol': _col(sp), 'cbpcol': _col(cbp),
        'Wpc': cc(Wpc), 'spccol': _col(spc), 'cbpccol': _col(cbpc),
        'W6a': cc(W6[0:128]), 'W6b': cc(W6[128:192]),
        's6col': _col(s6), 'cb6col': _col(cb6),
        'W7': cc(W7), 's7col': _col(s7), 'cb7col': _col(cb7),
        'ones3': np.ones((3, 1), np.float32),
    }
    in_maps = []
    for c in range(NC_):
        s = slice(c * M, (c + 1) * M)
        d = dict(rep)
        d['aT_m'] = cc(a[s].T)                                   # (64,128)
        d['a_m'] = cc(a[s])                                      # (128,64)
        d['peT_m'] = cc(pe[s].T)                                 # (64,128)
        d['wxyzT_m'] = cc(wxyz[s].T)                             # (3,128)
        d['wpT_m'] = cc(wp[s].T)                                 # (64,128)
        d['d2lhsT'] = cc(np.concatenate(
            [-2.0 * wxyz[s].T, sq[s][None, :],
             np.ones((1, M), np.float32)], 0))                   # (5,128)
        blob = np.zeros((128, CONST_COLS), np.float32)
        for nm, r, ccols in CONST_LAYOUT:
            t = d[nm]
            assert t.shape == (r, ccols), (nm, t.shape, (r, ccols))
            blob[0:r, CONST_OFF[nm]:CONST_OFF[nm] + ccols] = t
        import ml_dtypes
        bset = {
            'qeT_b': qe.T, 'rhsB2_b': np.concatenate([b.T, f2x.T], 0),
            'W1mid_b': W1[6:70], 'W1f2_b': W1[3:6],
            'W70bc_b': np.repeat(W1[70][None, :], 64, 0),
            'W71bc_b': np.repeat(W1[71][None, :], 64, 0),
            'W2dup_b': W2dup, 'W3d2_b': W3d2, 'W4_b': W4,
            'W4r_b': np.concatenate([W4[64:128], W4[0:64]], 0),
            'W5dup_b': W5dup,
            'Wpc_b': Wpc, 'W6a_b': W6[0:128], 'W6b_b': W6[128:192],
            'W7_b': W7,
            'ones512row_b': np.ones((1, 512), np.float32),
            'cb3duprow_b': np.concatenate([cb3, cb3])[None, :],
            'cb4row_b': cb4[None, :],
            'cb2duprow_b': np.concatenate([cb2, cb2])[None, :],
        }
        blob2 = np.zeros((128, BF16_COLS), ml_dtypes.bfloat16)
        for nm, r, ccols in BF16_LAYOUT:
            t = np.asarray(bset[nm], np.float32)
            assert t.shape == (r, ccols), (nm, t.shape, (r, ccols))
            blob2[0:r, BF16_OFF[nm]:BF16_OFF[nm] + ccols] = t.astype(
                ml_dtypes.bfloat16)
        in_maps.append({'CONST': blob, 'CONSTB': blob2,
                        'wxyz4': rep['wxyz4']})
    return in_maps


_SHAPES = None


def _build(timing_mode=False):
    nc = bacc.Bacc()
    P = {}
    P['CONST'] = nc.declare_dram_parameter('CONST', [128, CONST_COLS], f32,
                                           isOutput=False)
    P['CONSTB'] = nc.declare_dram_parameter('CONSTB', [128, BF16_COLS],
                                            bf16, isOutput=False)
    P['wxyz4'] = nc.declare_dram_parameter('wxyz4', [N, 4], f32,
                                           isOutput=False)
    out_p = nc.declare_dram_parameter("out", [M, 64], f32, isOutput=True)

    pf_mine = nc.dram_tensor("pf_mine", [M, 64], f32)
    pf_full = nc.dram_tensor("pf_full", [N, 64], f32, addr_space="Shared")

    with TileContext(nc) as tc:
        v = nc.vector
        sc = nc.scalar
        te = nc.tensor

        import contextlib
        est = contextlib.ExitStack()
        with est:
            const = est.enter_context(tc.tile_pool(name="const", bufs=1))

            cblk = const.tile([128, CONST_COLS], f32, tag="cblk")
            nc.sync.dma_start(out=cblk[:], in_=P['CONST'][:])

            def load(name):
                r = dict((n, rr) for n, rr, _ in CONST_LAYOUT)[name]
                c = dict((n, cc_) for n, _, cc_ in CONST_LAYOUT)[name]
                o = CONST_OFF[name]
                return cblk[0:r, o:o + c]

            cblk2 = const.tile([128, BF16_COLS], bf16, tag="cblk2")
            nc.sync.dma_start(out=cblk2[:], in_=P['CONSTB'][:])

            def loadb(name):
                r = dict((n, rr) for n, rr, _ in BF16_LAYOUT)[name]
                c = dict((n, cc_) for n, _, cc_ in BF16_LAYOUT)[name]
                o = BF16_OFF[name]
                return cblk2[0:r, o:o + c]

            # ---- static SBUF loads ----
            rhsA = load('rhsA')
            bT = load('bT')
            f2xT = load('f2xT')
            aT = load('aT')
            bsb = load('bsb')
            d2rhs = load('d2rhs')
            ident = load('ident')
            W1mid = loadb('W1mid_b'); W1f2 = loadb('W1f2_b')
            W70bc = loadb('W70bc_b'); W71bc = loadb('W71bc_b')
            W1xyz = load('W1xyz')
            qeTb = loadb('qeT_b'); rhsB2 = loadb('rhsB2_b')
            ones512row = loadb('ones512row_b')
            cb3duprow = loadb('cb3duprow_b'); cb4row = loadb('cb4row_b')
            cb2duprow = loadb('cb2duprow_b')
            s1col = load('s1col'); cb1col = load('cb1col')
            W2dup = loadb('W2dup_b'); s2dup = load('s2dup'); cb2dup = load('cb2dup')
            W3d2 = loadb('W3d2_b'); s3col = load('s3col'); cb3col = load('cb3col')
            W4 = loadb('W4_b'); W4r = loadb('W4r_b')
            s4col = load('s4col'); cb4col = load('cb4col')
            W5dup = loadb('W5dup_b'); s5dup = load('s5dup'); cb5dup = load('cb5dup')
            Wp36 = load('Wp36'); Wp03 = load('Wp03')
            spcol = load('spcol'); cbpcol = load('cbpcol')
            Wpc = loadb('Wpc_b'); spccol = load('spccol'); cbpccol = load('cbpccol')
            W6a = loadb('W6a_b'); W6b = loadb('W6b_b')
            s6col = load('s6col'); cb6col = load('cb6col')
            W7 = loadb('W7_b'); s7col = load('s7col'); cb7col = load('cb7col')
            ones3 = load('ones3')
            aT_m = load('aT_m'); a_m = load('a_m'); peT_m = load('peT_m')
            wxyzT_m = load('wxyzT_m'); wpT_m = load('wpT_m')
            d2lhsT = load('d2lhsT')

            rhsA2 = const.tile([128, 1024], bf16, tag="rhsA2")
            SVpT = const.tile([64, 1024], bf16, tag="SVpT")
            bias1T = const.tile([128, 128], f32, tag="bias1T")
            biaspT = const.tile([64, 128], f32, tag="biaspT")
            airT = const.tile([64, 128], f32, tag="airT")
            numT = const.tile([128, 64], f32, tag="numT")
            denT = const.tile([128, 64], f32, tag="denT")
            idxs = const.tile([128, 16], u32, tag="idxs")
            nd2 = const.tile([128, 1024], f32, tag="nd2")
            nd2b = const.tile([128, 1024], f32, tag="nd2b")
            gfT = const.tile([64, 2048], bf16, tag="gfT")
            pcrhs = const.tile([10, 2048], bf16, tag="pcrhs")
            ones128 = const.tile([128, 1024], bf16, tag="ones128")
            v.memset(ones128[:], 1.0)
            v.tensor_copy(out=rhsA2[0:64, :], in_=qeTb[:])

            # =========== prep ===========
            with tc.tile_pool(name="ppp", bufs=2, space="PSUM") as ppp, \
                 tc.tile_pool(name="pps", bufs=2) as pps:
                # bias1T = s1*(W1xyz^T @ wxyzT_m) + cb1
                u1p = ppp.tile([128, 128], f32, tag="pp")
                te.matmul(out=u1p[:], lhsT=W1xyz[:], rhs=wxyzT_m[:],
                          start=True, stop=True)
                v.tensor_scalar(out=bias1T[:], in0=u1p[:], scalar1=s1col[:],
                                scalar2=cb1col[:], op0=OP.mult, op1=OP.add)
                # biaspT = sp*(Wp03^T @ wxyzT_m) + cbp
                upp = ppp.tile([64, 128], f32, tag="pp")
                te.matmul(out=upp[:], lhsT=Wp03[:], rhs=wxyzT_m[:],
                          start=True, stop=True)
                v.tensor_scalar(out=biaspT[:], in0=upp[:], scalar1=spcol[:],
                                scalar2=cbpcol[:], op0=OP.mult, op1=OP.add)
                # SVpT = sp * (Wp36^T @ f2xT)
                for cch in range(2):
                    svp = ppp.tile([64, 512], f32, tag="pp")
                    te.matmul(out=svp[:], lhsT=Wp36[:],
                              rhs=f2xT[:, cch * 512:(cch + 1) * 512],
                              start=True, stop=True)
                    sc.activation(out=SVpT[:, cch * 512:(cch + 1) * 512],
                                  in_=svp[:], func=AF.Copy, scale=spcol[:])
                # rowmax of cos_mine -> ir -> airT
                rmx = pps.tile([128, 2], f32, tag="rmx")
                for cch in range(2):
                    cmp_ = ppp.tile([128, 512], f32, tag="pp")
                    te.matmul(out=cmp_[:], lhsT=aT_m[:],
                              rhs=bT[:, cch * 512:(cch + 1) * 512],
                              start=True, stop=True)
                    v.tensor_reduce(out=rmx[:, cch:cch + 1], in_=cmp_[:],
                                    axis=mybir.AxisListType.X, op=OP.max)
                irc = pps.tile([128, 1], f32, tag="irc")
                v.tensor_tensor(out=irc[:], in0=rmx[:, 0:1], in1=rmx[:, 1:2],
                                op=OP.max)
                v.tensor_scalar(out=irc[:], in0=irc[:], scalar1=1e-10,
                                scalar2=None, op0=OP.add)
                v.reciprocal(out=irc[:], in_=irc[:])
                air = pps.tile([128, 64], f32, tag="air")
                v.tensor_scalar(out=air[:], in0=a_m[:], scalar1=irc[:],
                                scalar2=None, op0=OP.mult)
                airp = ppp.tile([64, 128], f32, tag="pp")
                te.transpose(out=airp[:], in_=air[:], identity=ident[:])
                v.tensor_copy(out=airT[:], in_=airp[:])
                # colmax over all n per m-tile -> ic -> bic -> bicT (rhsB64)
                for t in range(8):
                    cmx = pps.tile([128, 2], f32, tag="cmx")
                    for cch in range(2):
                        ctp = ppp.tile([128, 512], f32, tag="pp")
                        te.matmul(out=ctp[:],
                                  lhsT=bT[:, t * 128:(t + 1) * 128],
                                  rhs=aT[:, cch * 512:(cch + 1) * 512],
                                  start=True, stop=True)
                        v.tensor_reduce(out=cmx[:, cch:cch + 1], in_=ctp[:],
                                        axis=mybir.AxisListType.X, op=OP.max)
                    icc = pps.tile([128, 1], f32, tag="icc")
                    v.tensor_tensor(out=icc[:], in0=cmx[:, 0:1],
                                    in1=cmx[:, 1:2], op=OP.max)
                    v.tensor_scalar(out=icc[:], in0=icc[:], scalar1=1e-10,
                                    scalar2=None, op0=OP.add)
                    v.reciprocal(out=icc[:], in_=icc[:])
                    bict = pps.tile([128, 64], f32, tag="bict")
                    v.tensor_scalar(out=bict[:],
                                    in0=bsb[:, t * 64:(t + 1) * 64],
                                    scalar1=icc[:], scalar2=None, op0=OP.mult)
                    bicp = ppp.tile([64, 128], f32, tag="pp")
                    te.transpose(out=bicp[:], in_=bict[:], identity=ident[:])
                    v.tensor_copy(out=rhsA2[64:128, t * 128:(t + 1) * 128],
                                  in_=bicp[:])
                # d2 -> nd2 -> top-16 indices
                d2p = ppp.tile([128, 1024], f32, tag="d2p")
                for cch in range(2):
                    te.matmul(out=d2p[:, cch * 512:(cch + 1) * 512],
                              lhsT=d2lhsT[:],
                              rhs=d2rhs[:, cch * 512:(cch + 1) * 512],
                              start=True, stop=True)
                sc.activation(out=nd2[:], in_=d2p[:], func=AF.Copy, scale=-1.0)
                mx8a = pps.tile([128, 8], f32, tag="mx8")
                v.max(out=mx8a[:], in_=nd2[:])
                v.max_index(out=idxs[:, 0:8], in_max=mx8a[:], in_values=nd2[:])
                v.match_replace(out=nd2b[:], in_to_replace=mx8a[:],
                                in_values=nd2[:], imm_value=-3.0e38)
                mx8b = pps.tile([128, 8], f32, tag="mx8")
                v.max(out=mx8b[:], in_=nd2b[:])
                v.max_index(out=idxs[:, 8:16], in_max=mx8b[:], in_values=nd2b[:])

            # =========== stage-1 loop over pairs of query points ===========
            with tc.tile_pool(name="psA", bufs=4, space="PSUM") as psA, \
                 tc.tile_pool(name="psB", bufs=2, space="PSUM") as psB, \
                 tc.tile_pool(name="lsb", bufs=4) as lsb:
                for i in range(M // 2):
                    nn = [2 * i, 2 * i + 1]
                    y1s = []
                    cats = []
                    h1s = []
                    for half, n in enumerate(nn):
                        # lA2: [pe*W1mid ; a*W71] vs rhsA2=[qeT; bicT]
                        lA = lsb.tile([128, 128], bf16, tag="lA")
                        v.tensor_scalar(out=lA[0:64, :], in0=W1mid[:],
                                        scalar1=peT_m[:, n:n + 1],
                                        scalar2=None, op0=OP.mult)
                        v.tensor_scalar(out=lA[64:128, :], in0=W71bc[:],
                                        scalar1=aT_m[:, n:n + 1],
                                        scalar2=None, op0=OP.mult)
                        # lB2: [ir*a*W70 ; W1f2] vs rhsB2=[bT; f2xT]
                        lB = lsb.tile([67, 128], bf16, tag="lB")
                        v.tensor_scalar(out=lB[0:64, :], in0=W70bc[:],
                                        scalar1=airT[:, n:n + 1],
                                        scalar2=None, op0=OP.mult)
                        v.tensor_copy(out=lB[64:67, :], in_=W1f2[:])
                        y1 = lsb.tile([128, 1024], bf16, tag=f"y1s{half}")
                        for cch in range(2):
                            cs = slice(cch * 512, (cch + 1) * 512)
                            y1p = psA.tile([128, 512], f32, tag="y1")
                            te.matmul(out=y1p[:], lhsT=lA[:],
                                      rhs=rhsA2[:, cs], start=True, stop=False)
                            te.matmul(out=y1p[:], lhsT=lB[:],
                                      rhs=rhsB2[:, cs], start=False, stop=True)
                            sc.activation(out=y1[:, cs], in_=y1p[:],
                                          func=AF.Relu,
                                          bias=bias1T[:, n:n + 1],
                                          scale=s1col[:])
                        y1s.append(y1)
                        cat = lsb.tile([128, 1024], bf16, tag=f"cat{half}")
                        pi_rows = slice(0, 64) if half == 0 else slice(64, 128)
                        v.tensor_scalar(
                            out=cat[pi_rows, :], in0=SVpT[:],
                            scalar1=biaspT[:, n:n + 1], scalar2=0.0,
                            op0=OP.add, op1=OP.max)
                        cats.append(cat)
                    # y2 (both points packed on partitions)
                    y2s = lsb.tile([128, 1024], bf16, tag="y2s")
                    for cch in range(2):
                        cs = slice(cch * 512, (cch + 1) * 512)
                        y2p = psB.tile([128, 512], f32, tag="midA")
                        te.matmul(out=y2p[0:64, :], lhsT=W2dup[:, 0:64],
                                  rhs=y1s[0][:, cs], start=True, stop=True)
                        te.matmul(out=y2p[64:128, :], lhsT=W2dup[:, 64:128],
                                  rhs=y1s[1][:, cs], start=True, stop=True)
                        sc.activation(out=y2s[:, cs], in_=y2p[:], func=AF.Relu,
                                      bias=cb2dup[:], scale=s2dup[:])
                    # y3
                    for cch in range(2):
                        cs = slice(cch * 512, (cch + 1) * 512)
                        y3p = psB.tile([128, 512], f32, tag="midA")
                        te.matmul(out=y3p[:], lhsT=cb3duprow[:],
                                  rhs=ones512row[:], start=True, stop=False)
                        te.matmul(out=y3p[0:64, :], lhsT=W3d2[0:64, 0:64],
                                  rhs=y2s[0:64, cs], start=False, stop=True)
                        te.matmul(out=y3p[64:128, :], lhsT=W3d2[64:128, 64:128],
                                  rhs=y2s[64:128, cs], start=False, stop=True,
                                  tile_position=(64, 64))
                        sc.activation(out=cats[0][64:128, cs], in_=y3p[0:64, :],
                                      func=AF.Relu, scale=s3col[:])
                        v.tensor_scalar(out=cats[1][0:64, cs],
                                        in0=y3p[64:128, :],
                                        scalar1=s3col[:], scalar2=0.0,
                                        op0=OP.mult, op1=OP.max)
                    # h1 per point
                    for half, n in enumerate(nn):
                        w4t = W4 if half == 0 else W4r
                        h1 = lsb.tile([128, 1024], bf16, tag=f"h1s{half}")
                        for cch in range(2):
                            cs = slice(cch * 512, (cch + 1) * 512)
                            h1p = psB.tile([128, 512], f32, tag="midB")
                            te.matmul(out=h1p[:], lhsT=cb4row[:],
                                      rhs=ones512row[:], start=True, stop=False)
                            te.matmul(out=h1p[:], lhsT=w4t[:],
                                      rhs=cats[half][:, cs],
                                      start=False, stop=True)
                            v.tensor_scalar(out=h1[:, cs], in0=h1p[:],
                                            scalar1=s4col[:], scalar2=0.0,
                                            op0=OP.mult, op1=OP.max)
                        h1s.append(h1)
                    # h -> exp
                    texp = lsb.tile([128, 1024], bf16, tag="texp")
                    for cch in range(2):
                        cs = slice(cch * 512, (cch + 1) * 512)
                        hp = psB.tile([128, 512], f32, tag="midB")
                        te.matmul(out=hp[0:64, :], lhsT=W5dup[:, 0:64],
                                  rhs=h1s[1][:, cs], start=True, stop=True)
                        te.matmul(out=hp[64:128, :], lhsT=W5dup[:, 64:128],
                                  rhs=h1s[0][:, cs], start=True, stop=True)
                        sc.activation(out=texp[:, cs], in_=hp[:], func=AF.Exp,
                                      bias=cb5dup[:], scale=s5dup[:])
                    # num/den with fused relu: E = max(exp(z),1)
                    scr = lsb.tile([128, 1024], bf16, tag="scr")
                    v.scalar_tensor_tensor(
                        out=scr[64:128, :], in0=texp[64:128, :], scalar=1.0,
                        in1=cats[0][64:128, :], op0=OP.max, op1=OP.mult,
                        accum_out=numT[64:128, i:i + 1])
                    v.scalar_tensor_tensor(
                        out=scr[0:64, :], in0=texp[0:64, :], scalar=1.0,
                        in1=cats[1][0:64, :], op0=OP.max, op1=OP.mult,
                        accum_out=numT[0:64, i:i + 1])
                    scr2 = lsb.tile([128, 1024], bf16, tag="scr2")
                    v.tensor_scalar(
                        out=scr2[:], in0=texp[:], scalar1=1.0, scalar2=0.0,
                        op0=OP.max, op1=OP.add,
                        accum_out=denT[:, i:i + 1])

            # =========== pi_feat assembly + AllGather ===========
            with tc.tile_pool(name="ps2", bufs=2, space="PSUM") as ps2, \
                 tc.tile_pool(name="sb2", bufs=1) as sb2, \
                 tc.tile_pool(name="gsb", bufs=3) as gsb:
                denR = sb2.tile([128, 64], f32, tag="denR")
                v.reciprocal(out=denR[:], in_=denT[:])
                pfZ = sb2.tile([128, 64], f32, tag="pfZ")
                v.tensor_tensor(out=pfZ[:], in0=numT[:], in1=denR[:],
                                op=OP.mult)
                pfZ2 = sb2.tile([128, 64], f32, tag="pfZ2")
                v.tensor_copy(out=pfZ2[0:64, :], in_=pfZ[64:128, :])
                v.tensor_copy(out=pfZ2[64:128, :], in_=pfZ[0:64, :])
                pfp = ps2.tile([64, 128], f32, tag="tp")
                te.transpose(out=pfp[:], in_=pfZ2[:], identity=ident[:])
                pfs = sb2.tile([64, 128], f32, tag="pfs")
                v.tensor_copy(out=pfs[:], in_=pfp[:])
                nc.sync.dma_start(
                    out=pf_mine[:].rearrange("(i h) j -> i (h j)", h=2),
                    in_=pfs[:])
                if timing_mode:
                    nc.sync.dma_start(out=pf_full[0:M, :], in_=pf_mine[:])
                else:
                    nc.gpsimd.collective_compute(
                        "AllGather", OP.bypass,
                        replica_groups=[list(range(NC_))],
                        ins=[pf_mine[:]],
                        outs=[pf_full[:]],
                    )
                gxTt = sb2.tile([3, 2048], bf16, tag="gxTt")
                wbc = sb2.tile([3, 2048], bf16, tag="wbc")
                # gathers + transposes into channel-major
                for k in range(KNN):
                    gf = gsb.tile([128, 64], f32, tag="gf")
                    nc.gpsimd.indirect_dma_start(
                        out=gf[:], out_offset=None, in_=pf_full[:],
                        in_offset=bass.IndirectOffsetOnAxis(
                            ap=idxs[:, k:k + 1], axis=0))
                    gfp = ps2.tile([64, 128], f32, tag="tp")
                    te.transpose(out=gfp[:], in_=gf[:], identity=ident[:])
                    v.tensor_copy(out=gfT[:, k * 128:(k + 1) * 128], in_=gfp[:])
                    gx = gsb.tile([128, 4], f32, tag="gx")
                    nc.gpsimd.indirect_dma_start(
                        out=gx[:], out_offset=None, in_=P['wxyz4'][:],
                        in_offset=bass.IndirectOffsetOnAxis(
                            ap=idxs[:, k:k + 1], axis=0))
                    gxp = ps2.tile([4, 128], f32, tag="tp4")
                    te.transpose(out=gxp[:], in_=gx[:], identity=ident[:])
                    v.tensor_copy(out=gxTt[:, k * 128:(k + 1) * 128],
                                  in_=gxp[0:3, :])
                    v.tensor_copy(out=wbc[:, k * 128:(k + 1) * 128],
                                  in_=wxyzT_m[:])

                # pc10 channels: [wxyz, g_xyz, diff, euc]
                v.tensor_copy(out=pcrhs[0:3, :], in_=wbc[:])
                nc.sync.dma_start(out=pcrhs[3:6, :], in_=gxTt[:])
                dft = sb2.tile([3, 2048], bf16, tag="dft")
                v.tensor_tensor(out=dft[:], in0=gxTt[:], in1=wbc[:],
                                op=OP.subtract)
                nc.sync.dma_start(out=pcrhs[6:9, :], in_=dft[:])
                ones3b = sb2.tile([3, 1], bf16, tag="ones3b")
                v.memset(ones3b[:], 1.0)
                dsq = sb2.tile([3, 2048], bf16, tag="dsq")
                v.tensor_tensor(out=dsq[:], in0=dft[:], in1=dft[:],
                                op=OP.mult)
                euct = sb2.tile([1, 2048], bf16, tag="euct")
                for cch in range(4):
                    cs = slice(cch * 512, (cch + 1) * 512)
                    ecp = ps2.tile([1, 512], f32, tag="sm")
                    te.matmul(out=ecp[:], lhsT=ones3b[:], rhs=dsq[:, cs],
                              start=True, stop=True)
                    sc.activation(out=euct[:, cs], in_=ecp[:], func=AF.Sqrt)
                nc.sync.dma_start(out=pcrhs[9:10, :], in_=euct[:])
                # pc_enc
                rhs2a = sb2.tile([128, 2048], bf16, tag="rhs2a")
                for cch in range(4):
                    cs = slice(cch * 512, (cch + 1) * 512)
                    pcp = ps2.tile([64, 512], f32, tag="big")
                    te.matmul(out=pcp[:], lhsT=Wpc[:], rhs=pcrhs[:, cs],
                              start=True, stop=True)
                    sc.activation(out=rhs2a[0:64, cs], in_=pcp[:],
                                  func=AF.Relu, bias=cbpccol[:],
                                  scale=spccol[:])
                for k in range(KNN):
                    v.tensor_copy(out=rhs2a[64:128, k * 128:(k + 1) * 128],
                                  in_=wpT_m[:])
                # mlp2b layer 1
                h1bs = sb2.tile([128, 2048], bf16, tag="h1bs")
                for cch in range(4):
                    cs = slice(cch * 512, (cch + 1) * 512)
                    h1bp = ps2.tile([128, 512], f32, tag="big")
                    te.matmul(out=h1bp[:], lhsT=W6a[:], rhs=rhs2a[:, cs],
                              start=True, stop=False)
                    te.matmul(out=h1bp[:], lhsT=W6b[:], rhs=gfT[:, cs],
                              start=False, stop=True)
                    sc.activation(out=h1bs[:, cs], in_=h1bp[:], func=AF.Relu,
                                  bias=cb6col[:], scale=s6col[:])
                # mlp2b layer 2 -> exp
                texp2 = sb2.tile([64, 2048], bf16, tag="texp2")
                for cch in range(4):
                    cs = slice(cch * 512, (cch + 1) * 512)
                    z2p = ps2.tile([64, 512], f32, tag="big")
                    te.matmul(out=z2p[:], lhsT=W7[:], rhs=h1bs[:, cs],
                              start=True, stop=True)
                    sc.activation(out=texp2[:, cs], in_=z2p[:], func=AF.Exp,
                                  bias=cb7col[:], scale=s7col[:])
                # num2 / den2 with segment reduce over k (free stride 128)
                scr2a = sb2.tile([64, 2048], bf16, tag="scr2a")
                v.scalar_tensor_tensor(out=scr2a[:], in0=texp2[:], scalar=1.0,
                                       in1=gfT[:], op0=OP.max, op1=OP.mult)
                num2 = sb2.tile([64, 128], f32, tag="num2")
                v.tensor_reduce(
                    out=num2[:],
                    in_=scr2a[:].rearrange("p (k n) -> p n k", k=KNN),
                    axis=mybir.AxisListType.X, op=OP.add)
                scr2b = sb2.tile([64, 2048], bf16, tag="scr2b")
                v.tensor_scalar(out=scr2b[:], in0=texp2[:], scalar1=1.0,
                                scalar2=None, op0=OP.max)
                den2 = sb2.tile([64, 128], f32, tag="den2")
                v.tensor_reduce(
                    out=den2[:],
                    in_=scr2b[:].rearrange("p (k n) -> p n k", k=KNN),
                    axis=mybir.AxisListType.X, op=OP.add)
                den2r = sb2.tile([64, 128], f32, tag="den2r")
                v.reciprocal(out=den2r[:], in_=den2[:])
                outT = sb2.tile([64, 128], f32, tag="outT")
                v.tensor_tensor(out=outT[:], in0=num2[:], in1=den2r[:],
                                op=OP.mult)
                outp = ps2.tile([128, 64], f32, tag="tp")
                te.transpose(out=outp[:], in_=outT[:],
                             identity=ident[0:64, 0:64])
                outs = sb2.tile([128, 64], f32, tag="outs")
                v.tensor_copy(out=outs[:], in_=outp[:])
                nc.sync.dma_start(out=out_p[:], in_=outs[:])

    nc.finalize()
    return nc


_NC_CACHE = None


def kernel(**inputs):
    global LAST_EXEC_NS, LAST_RESULT, _NC_CACHE
    in_maps = _host_prep(inputs)
    if _NC_CACHE is None:
        _NC_CACHE = _build()
    import os
    res = run_bass_kernel_spmd(
        _NC_CACHE, in_maps, core_ids=list(range(NC_)),
        trace=bool(os.environ.get("KERNEL_TRACE")),
    )
    LAST_RESULT = res
    LAST_EXEC_NS = res.exec_time_ns
    out = np.concatenate([res.results[i]["out"] for i in range(NC_)], 0)
    return out[None].astype(np.float32)


# revision 32
# speedup vs baseline: 1.1912x; 1.0606x over previous
"""AllCostVolume Trainium2 kernel: 8-core SPMD, query-point sharded.

Stage 1 (per query point n, fused in SBUF):
  72-channel pairwise feature MLP1 -> attention over N2 -> pi_feat,
  computed via a K=195 contraction decomposition: the pe*qe / rm / cm
  feature channels are folded into per-n stationary weights (DVE-built)
  against static rhs matrices [qeT; bT], [bicT], [f2xT].
Stage 2: KNN top-16 via max8/match_replace, indirect-DMA gathers,
  small MLP over (n,k) pairs, softmax over k.
Cross-core: AllGather of pi_feat (128x64 per core).
"""
import numpy as np

from concourse import bass, bacc, mybir
from concourse.tile import TileContext
from concourse.bass_utils import run_bass_kernel_spmd

f32 = mybir.dt.float32
bf16 = mybir.dt.bfloat16
u32 = mybir.dt.uint32
AF = mybir.ActivationFunctionType
OP = mybir.AluOpType

NC_ = 8          # cores
N = 1024         # query points
M = N // NC_     # per-core query points = 128
N2 = 1024
C = 64
KNN = 16
BN = 1.0 / np.sqrt(1.0 + 1e-5)

LAST_EXEC_NS = None
LAST_RESULT = None



# (name, rows, cols) — packed into one (128 x NCOLS) constant block
CONST_LAYOUT = [
    ('rhsA', 128, 1024), ('bT', 64, 1024), ('f2xT', 3, 1024),
    ('aT', 64, 1024), ('bsb', 128, 512), ('d2rhs', 5, 1024),
    ('ident', 128, 128),
    ('W1mid', 64, 128), ('W1f2', 3, 128), ('W70bc', 64, 128),
    ('W71bc', 64, 128), ('W1xyz', 3, 128),
    ('W2dup', 128, 128), ('W3d2', 128, 128), ('W4', 128, 128),
    ('W4r', 128, 128), ('W5dup', 128, 128),
    ('Wp36', 3, 64), ('Wp03', 3, 64), ('Wpc', 10, 64),
    ('W6a', 128, 128), ('W6b', 64, 128), ('W7', 128, 64),
    ('ones3', 3, 1),
    ('s1col', 128, 1), ('cb1col', 128, 1), ('s2dup', 128, 1),
    ('cb2dup', 128, 1), ('s3col', 64, 1), ('cb3col', 64, 1),
    ('s4col', 128, 1), ('cb4col', 128, 1), ('s5dup', 128, 1),
    ('cb5dup', 128, 1), ('spcol', 64, 1), ('cbpcol', 64, 1),
    ('spccol', 64, 1), ('cbpccol', 64, 1), ('s6col', 128, 1),
    ('cb6col', 128, 1), ('s7col', 64, 1), ('cb7col', 64, 1),
    ('aT_m', 64, 128), ('a_m', 128, 64), ('peT_m', 64, 128),
    ('wxyzT_m', 3, 128), ('wpT_m', 64, 128), ('d2lhsT', 5, 128),
]
BF16_LAYOUT = [
    ('qeT_b', 64, 1024), ('rhsB2_b', 67, 1024),
    ('W1mid_b', 64, 128), ('W1f2_b', 3, 128), ('W70bc_b', 64, 128),
    ('W71bc_b', 64, 128),
    ('W2dup_b', 128, 128), ('W3d2_b', 128, 128), ('W4_b', 128, 128),
    ('W4r_b', 128, 128), ('W5dup_b', 128, 128),
    ('Wpc_b', 10, 64), ('W6a_b', 128, 128), ('W6b_b', 64, 128),
    ('W7_b', 128, 64),
    ('ones512row_b', 1, 512), ('cb3duprow_b', 1, 128), ('cb4row_b', 1, 128),
    ('cb2duprow_b', 1, 128),
]
BF16_COLS = sum(c for _, _, c in BF16_LAYOUT)
BF16_OFF = {}
_o2 = 0
for _n, _r, _c in BF16_LAYOUT:
    BF16_OFF[_n] = _o2
    _o2 += _c

CONST_COLS = sum(c for _, _, c in CONST_LAYOUT)
CONST_OFF = {}
_o = 0
for _n, _r, _c in CONST_LAYOUT:
    CONST_OFF[_n] = _o
    _o += _c


def _fold(p):
    W, b, g, be = [np.asarray(x, np.float32) for x in p]
    s = (g * BN).astype(np.float32)
    cb = (b * s + be).astype(np.float32)
    return W, s, cb


def _col(x):
    return np.ascontiguousarray(np.asarray(x, np.float32).reshape(-1, 1))


def _host_prep(inputs):
    wxyz_in = np.asarray(inputs['warped_xyz'][0], np.float32)
    wp = np.asarray(inputs['warped_points'][0], np.float32)
    f2x = np.asarray(inputs['f2_xyz'][0], np.float32)
    f2p = np.asarray(inputs['f2_points'][0], np.float32)
    lz = np.asarray(inputs['lidar_z'][0], np.float32)

    W1, s1, cb1 = _fold(inputs['mlp1_params'][0])
    W2, s2, cb2 = _fold(inputs['mlp1_params'][1])
    W3, s3, cb3 = _fold(inputs['mlp1_params'][2])
    W4, s4, cb4 = _fold(inputs['mlp2_params'][0])
    W5, s5, cb5 = _fold(inputs['mlp2_params'][1])
    Wp, sp, cbp = _fold(inputs['pi_enc'])
    Wpc, spc, cbpc = _fold(inputs['pc_enc'])
    W6, s6, cb6 = _fold(inputs['mlp2b_params'][0])
    W7, s7, cb7 = _fold(inputs['mlp2b_params'][1])

    wxyz = (wxyz_in * lz).astype(np.float32)                     # (1024,3)
    a = wp / np.linalg.norm(wp, axis=1, keepdims=True)
    b = f2p / np.linalg.norm(f2p, axis=1, keepdims=True)
    pe = (wp - wp.mean(1, keepdims=True)) / wp.std(1, keepdims=True, ddof=1)
    qe = (f2p - f2p.mean(1, keepdims=True)) / f2p.std(1, keepdims=True, ddof=1)
    sq = (wxyz * wxyz).sum(1).astype(np.float32)

    cc = np.ascontiguousarray
    W2dup = cc(np.concatenate([W2, W2], 1))                      # (128,128)
    W3d2 = np.zeros((128, 128), np.float32)
    W3d2[0:64, 0:64] = W3
    W3d2[64:128, 64:128] = W3
    W5dup = cc(np.concatenate([W5, W5], 1))                      # (128,128)

    rep = {
        'rhsA': cc(np.concatenate([qe.T, b.T], 0)),
        'bT': cc(b.T),              # (128,1024)
        'f2xT': cc(f2x.T),                                       # (3,1024)
        'aT': cc(a.T),                                           # (64,1024)
        'bsb': cc(b.reshape(8, 128, 64).transpose(1, 0, 2).reshape(128, 512)),
        'd2rhs': cc(np.concatenate([wxyz.T, np.ones((1, N), np.float32),
                                    sq[None, :]], 0)),           # (5,1024)
        'wxyz4': cc(np.concatenate([wxyz, np.zeros((N, 1), np.float32)], 1)),
        'ident': np.eye(128, dtype=np.float32),
        'W1mid': cc(W1[6:70]), 'W1f2': cc(W1[3:6]),
        'W70bc': cc(np.repeat(W1[70][None, :], 64, 0)),
        'W71bc': cc(np.repeat(W1[71][None, :], 64, 0)),
        'W1xyz': cc(W1[0:3]),
        's1col': _col(s1), 'cb1col': _col(cb1),
        'W2dup': W2dup, 's2dup': _col(np.concatenate([s2, s2])),
        'cb2dup': _col(np.concatenate([cb2, cb2])),
        'W3d2': W3d2, 's3col': _col(s3), 'cb3col': _col(cb3),
        'W4': cc(W4), 's4col': _col(s4), 'cb4col': _col(cb4),
        'W4r': cc(np.concatenate([W4[64:128], W4[0:64]], 0)),
        'W5dup': W5dup, 's5dup': _col(np.concatenate([s5, s5])),
        'cb5dup': _col(np.concatenate([cb5, cb5])),
        'Wp36': cc(Wp[3:6]), 'Wp03': cc(Wp[0:3]),
        'spcol': _col(sp), 'cbpcol': _col(cbp),
        'Wpc': cc(Wpc), 'spccol': _col(spc), 'cbpccol': _col(cbpc),
        'W6a': cc(W6[0:128]), 'W6b': cc(W6[128:192]),
        's6col': _col(s6), 'cb6col': _col(cb6),
        'W7': cc(W7), 's7col': _col(s7), 'cb7col': _col(cb7),
        'ones3': np.ones((3, 1), np.float32),
    }
    in_maps = []
    for c in range(NC_):
        s = slice(c * M, (c + 1) * M)
        d = dict(rep)
        d['aT_m'] = cc(a[s].T)                                   # (64,128)
        d['a_m'] = cc(a[s])                                      # (128,64)
        d['peT_m'] = cc(pe[s].T)                                 # (64,128)
        d['wxyzT_m'] = cc(wxyz[s].T)                             # (3,128)
        d['wpT_m'] = cc(wp[s].T)                                 # (64,128)
        d['d2lhsT'] = cc(np.concatenate(
            [-2.0 * wxyz[s].T, sq[s][None, :],
             np.ones((1, M), np.float32)], 0))                   # (5,128)
        blob = np.zeros((128, CONST_COLS), np.float32)
        for nm, r, ccols in CONST_LAYOUT:
            t = d[nm]
            assert t.shape == (r, ccols), (nm, t.shape, (r, ccols))
            blob[0:r, CONST_OFF[nm]:CONST_OFF[nm] + ccols] = t
        import ml_dtypes
        bset = {
            'qeT_b': qe.T, 'rhsB2_b': np.concatenate([b.T, f2x.T], 0),
            'W1mid_b': W1[6:70], 'W1f2_b': W1[3:6],
            'W70bc_b': np.repeat(W1[70][None, :], 64, 0),
            'W71bc_b': np.repeat(W1[71][None, :], 64, 0),
            'W2dup_b': W2dup, 'W3d2_b': W3d2, 'W4_b': W4,
            'W4r_b': np.concatenate([W4[64:128], W4[0:64]], 0),
            'W5dup_b': W5dup,
            'Wpc_b': Wpc, 'W6a_b': W6[0:128], 'W6b_b': W6[128:192],
            'W7_b': W7,
            'ones512row_b': np.ones((1, 512), np.float32),
            'cb3duprow_b': np.concatenate([cb3, cb3])[None, :],
            'cb4row_b': cb4[None, :],
            'cb2duprow_b': np.concatenate([cb2, cb2])[None, :],
        }
        blob2 = np.zeros((128, BF16_COLS), ml_dtypes.bfloat16)
        for nm, r, ccols in BF16_LAYOUT:
            t = np.asarray(bset[nm], np.float32)
            assert t.shape == (r, ccols), (nm, t.shape, (r, ccols))
            blob2[0:r, BF16_OFF[nm]:BF16_OFF[nm] + ccols] = t.astype(
                ml_dtypes.bfloat16)
        in_maps.append({'CONST': blob, 'CONSTB': blob2,
                        'wxyz4': rep['wxyz4']})
    return in_maps


_SHAPES = None


def _build(timing_mode=False):
    nc = bacc.Bacc()
    P = {}
    P['CONST'] = nc.declare_dram_parameter('CONST', [128, CONST_COLS], f32,
                                           isOutput=False)
    P['CONSTB'] = nc.declare_dram_parameter('CONSTB', [128, BF16_COLS],
                                            bf16, isOutput=False)
    P['wxyz4'] = nc.declare_dram_parameter('wxyz4', [N, 4], f32,
                                           isOutput=False)
    out_p = nc.declare_dram_parameter("out", [M, 64], f32, isOutput=True)

    pf_mine = nc.dram_tensor("pf_mine", [M, 64], f32)
    pf_full = nc.dram_tensor("pf_full", [N, 64], f32, addr_space="Shared")

    with TileContext(nc) as tc:
        v = nc.vector
        sc = nc.scalar
        te = nc.tensor

        import contextlib
        est = contextlib.ExitStack()
        with est:
            const = est.enter_context(tc.tile_pool(name="const", bufs=1))

            cblk = const.tile([128, CONST_COLS], f32, tag="cblk")
            nc.sync.dma_start(out=cblk[:], in_=P['CONST'][:])

            def load(name):
                r = dict((n, rr) for n, rr, _ in CONST_LAYOUT)[name]
                c = dict((n, cc_) for n, _, cc_ in CONST_LAYOUT)[name]
                o = CONST_OFF[name]
                return cblk[0:r, o:o + c]

            cblk2 = const.tile([128, BF16_COLS], bf16, tag="cblk2")
            nc.sync.dma_start(out=cblk2[:], in_=P['CONSTB'][:])

            def loadb(name):
                r = dict((n, rr) for n, rr, _ in BF16_LAYOUT)[name]
                c = dict((n, cc_) for n, _, cc_ in BF16_LAYOUT)[name]
                o = BF16_OFF[name]
                return cblk2[0:r, o:o + c]

            # ---- static SBUF loads ----
            rhsA = load('rhsA')
            bT = load('bT')
            f2xT = load('f2xT')
            aT = load('aT')
            bsb = load('bsb')
            d2rhs = load('d2rhs')
            ident = load('ident')
            W1mid = loadb('W1mid_b'); W1f2 = loadb('W1f2_b')
            W70bc = loadb('W70bc_b'); W71bc = loadb('W71bc_b')
            W1xyz = load('W1xyz')
            qeTb = loadb('qeT_b'); rhsB2 = loadb('rhsB2_b')
            ones512row = loadb('ones512row_b')
            cb3duprow = loadb('cb3duprow_b'); cb4row = loadb('cb4row_b')
            cb2duprow = loadb('cb2duprow_b')
            s1col = load('s1col'); cb1col = load('cb1col')
            W2dup = loadb('W2dup_b'); s2dup = load('s2dup'); cb2dup = load('cb2dup')
            W3d2 = loadb('W3d2_b'); s3col = load('s3col'); cb3col = load('cb3col')
            W4 = loadb('W4_b'); W4r = loadb('W4r_b')
            s4col = load('s4col'); cb4col = load('cb4col')
            W5dup = loadb('W5dup_b'); s5dup = load('s5dup'); cb5dup = load('cb5dup')
            Wp36 = load('Wp36'); Wp03 = load('Wp03')
            spcol = load('spcol'); cbpcol = load('cbpcol')
            Wpc = loadb('Wpc_b'); spccol = load('spccol'); cbpccol = load('cbpccol')
            W6a = loadb('W6a_b'); W6b = loadb('W6b_b')
            s6col = load('s6col'); cb6col = load('cb6col')
            W7 = loadb('W7_b'); s7col = load('s7col'); cb7col = load('cb7col')
            ones3 = load('ones3')
            aT_m = load('aT_m'); a_m = load('a_m'); peT_m = load('peT_m')
            wxyzT_m = load('wxyzT_m'); wpT_m = load('wpT_m')
            d2lhsT = load('d2lhsT')

            rhsA2 = const.tile([128, 1024], bf16, tag="rhsA2")
            SVpT = const.tile([64, 1024], bf16, tag="SVpT")
            bias1T = const.tile([128, 128], f32, tag="bias1T")
            biaspT = const.tile([64, 128], f32, tag="biaspT")
            airT = const.tile([64, 128], f32, tag="airT")
            numT = const.tile([128, 64], f32, tag="numT")
            denT = const.tile([128, 64], f32, tag="denT")
            idxs = const.tile([128, 16], u32, tag="idxs")
            nd2 = const.tile([128, 1024], f32, tag="nd2")
            nd2b = const.tile([128, 1024], f32, tag="nd2b")
            gfT = const.tile([64, 2048], bf16, tag="gfT")
            pcrhs = const.tile([10, 2048], bf16, tag="pcrhs")
            ones128 = const.tile([128, 1024], bf16, tag="ones128")
            v.memset(ones128[:], 1.0)
            v.tensor_copy(out=rhsA2[0:64, :], in_=qeTb[:])

            # =========== prep ===========
            with tc.tile_pool(name="ppp", bufs=2, space="PSUM") as ppp, \
                 tc.tile_pool(name="pps", bufs=2) as pps:
                # bias1T = s1*(W1xyz^T @ wxyzT_m) + cb1
                u1p = ppp.tile([128, 128], f32, tag="pp")
                te.matmul(out=u1p[:], lhsT=W1xyz[:], rhs=wxyzT_m[:],
                          start=True, stop=True)
                v.tensor_scalar(out=bias1T[:], in0=u1p[:], scalar1=s1col[:],
                                scalar2=cb1col[:], op0=OP.mult, op1=OP.add)
                # biaspT = sp*(Wp03^T @ wxyzT_m) + cbp
                upp = ppp.tile([64, 128], f32, tag="pp")
                te.matmul(out=upp[:], lhsT=Wp03[:], rhs=wxyzT_m[:],
                          start=True, stop=True)
                v.tensor_scalar(out=biaspT[:], in0=upp[:], scalar1=spcol[:],
                                scalar2=cbpcol[:], op0=OP.mult, op1=OP.add)
                # SVpT = sp * (Wp36^T @ f2xT)
                for cch in range(2):
                    svp = ppp.tile([64, 512], f32, tag="pp")
                    te.matmul(out=svp[:], lhsT=Wp36[:],
                              rhs=f2xT[:, cch * 512:(cch + 1) * 512],
                              start=True, stop=True)
                    sc.activation(out=SVpT[:, cch * 512:(cch + 1) * 512],
                                  in_=svp[:], func=AF.Copy, scale=spcol[:])
                # rowmax of cos_mine -> ir -> airT
                rmx = pps.tile([128, 2], f32, tag="rmx")
                for cch in range(2):
                    cmp_ = ppp.tile([128, 512], f32, tag="pp")
                    te.matmul(out=cmp_[:], lhsT=aT_m[:],
                              rhs=bT[:, cch * 512:(cch + 1) * 512],
                              start=True, stop=True)
                    v.tensor_reduce(out=rmx[:, cch:cch + 1], in_=cmp_[:],
                                    axis=mybir.AxisListType.X, op=OP.max)
                irc = pps.tile([128, 1], f32, tag="irc")
                v.tensor_tensor(out=irc[:], in0=rmx[:, 0:1], in1=rmx[:, 1:2],
                                op=OP.max)
                v.tensor_scalar(out=irc[:], in0=irc[:], scalar1=1e-10,
                                scalar2=None, op0=OP.add)
                v.reciprocal(out=irc[:], in_=irc[:])
                air = pps.tile([128, 64], f32, tag="air")
                v.tensor_scalar(out=air[:], in0=a_m[:], scalar1=irc[:],
                                scalar2=None, op0=OP.mult)
                airp = ppp.tile([64, 128], f32, tag="pp")
                te.transpose(out=airp[:], in_=air[:], identity=ident[:])
                v.tensor_copy(out=airT[:], in_=airp[:])
                # colmax over all n per m-tile -> ic -> bic -> bicT (rhsB64)
                for t in range(8):
                    cmx = pps.tile([128, 2], f32, tag="cmx")
                    for cch in range(2):
                        ctp = ppp.tile([128, 512], f32, tag="pp")
                        te.matmul(out=ctp[:],
                                  lhsT=bT[:, t * 128:(t + 1) * 128],
                                  rhs=aT[:, cch * 512:(cch + 1) * 512],
                                  start=True, stop=True)
                        v.tensor_reduce(out=cmx[:, cch:cch + 1], in_=ctp[:],
                                        axis=mybir.AxisListType.X, op=OP.max)
                    icc = pps.tile([128, 1], f32, tag="icc")
                    v.tensor_tensor(out=icc[:], in0=cmx[:, 0:1],
                                    in1=cmx[:, 1:2], op=OP.max)
                    v.tensor_scalar(out=icc[:], in0=icc[:], scalar1=1e-10,
                                    scalar2=None, op0=OP.add)
                    v.reciprocal(out=icc[:], in_=icc[:])
                    bict = pps.tile([128, 64], f32, tag="bict")
                    v.tensor_scalar(out=bict[:],
                                    in0=bsb[:, t * 64:(t + 1) * 64],
                                    scalar1=icc[:], scalar2=None, op0=OP.mult)
                    bicp = ppp.tile([64, 128], f32, tag="pp")
                    te.transpose(out=bicp[:], in_=bict[:], identity=ident[:])
                    v.tensor_copy(out=rhsA2[64:128, t * 128:(t + 1) * 128],
                                  in_=bicp[:])
                # d2 -> nd2 -> top-16 indices
                d2p = ppp.tile([128, 1024], f32, tag="d2p")
                for cch in range(2):
                    te.matmul(out=d2p[:, cch * 512:(cch + 1) * 512],
                              lhsT=d2lhsT[:],
                              rhs=d2rhs[:, cch * 512:(cch + 1) * 512],
                              start=True, stop=True)
                sc.activation(out=nd2[:], in_=d2p[:], func=AF.Copy, scale=-1.0)
                mx8a = pps.tile([128, 8], f32, tag="mx8")
                v.max(out=mx8a[:], in_=nd2[:])
                v.max_index(out=idxs[:, 0:8], in_max=mx8a[:], in_values=nd2[:])
                v.match_replace(out=nd2b[:], in_to_replace=mx8a[:],
                                in_values=nd2[:], imm_value=-3.0e38)
                mx8b = pps.tile([128, 8], f32, tag="mx8")
                v.max(out=mx8b[:], in_=nd2b[:])
                v.max_index(out=idxs[:, 8:16], in_max=mx8b[:], in_values=nd2b[:])

            # =========== stage-1 loop over pairs of query points ===========
            with tc.tile_pool(name="psA", bufs=4, space="PSUM") as psA, \
                 tc.tile_pool(name="psB", bufs=2, space="PSUM") as psB, \
                 tc.tile_pool(name="lsb", bufs=4) as lsb:
                for i in range(M // 2):
                    nn = [2 * i, 2 * i + 1]
                    y1s = []
                    cats = []
                    h1s = []
                    for half, n in enumerate(nn):
                        # lA2: [pe*W1mid ; a*W71] vs rhsA2=[qeT; bicT]
                        lA = lsb.tile([128, 128], bf16, tag="lA")
                        v.tensor_scalar(out=lA[0:64, :], in0=W1mid[:],
                                        scalar1=peT_m[:, n:n + 1],
                                        scalar2=None, op0=OP.mult)
                        v.tensor_scalar(out=lA[64:128, :], in0=W71bc[:],
                                        scalar1=aT_m[:, n:n + 1],
                                        scalar2=None, op0=OP.mult)
                        # lB2: [ir*a*W70 ; W1f2] vs rhsB2=[bT; f2xT]
                        lB = lsb.tile([67, 128], bf16, tag="lB")
                        v.tensor_scalar(out=lB[0:64, :], in0=W70bc[:],
                                        scalar1=airT[:, n:n + 1],
                                        scalar2=None, op0=OP.mult)
                        v.tensor_copy(out=lB[64:67, :], in_=W1f2[:])
                        y1 = lsb.tile([128, 1024], bf16, tag=f"y1s{half}")
                        for cch in range(2):
                            cs = slice(cch * 512, (cch + 1) * 512)
                            y1p = psA.tile([128, 512], f32, tag="y1")
                            te.matmul(out=y1p[:], lhsT=lA[:],
                                      rhs=rhsA2[:, cs], start=True, stop=False)
                            te.matmul(out=y1p[:], lhsT=lB[:],
                                      rhs=rhsB2[:, cs], start=False, stop=True)
                            sc.activation(out=y1[:, cs], in_=y1p[:],
                                          func=AF.Relu,
                                          bias=bias1T[:, n:n + 1],
                                          scale=s1col[:])
                        y1s.append(y1)
                        cat = lsb.tile([128, 1024], bf16, tag=f"cat{half}")
                        pi_rows = slice(0, 64) if half == 0 else slice(64, 128)
                        v.tensor_scalar(
                            out=cat[pi_rows, :], in0=SVpT[:],
                            scalar1=biaspT[:, n:n + 1], scalar2=0.0,
                            op0=OP.add, op1=OP.max)
                        cats.append(cat)
                    # y2 (both points packed on partitions)
                    y2s = lsb.tile([128, 1024], bf16, tag="y2s")
                    for cch in range(2):
                        cs = slice(cch * 512, (cch + 1) * 512)
                        y2p = psB.tile([128, 512], f32, tag="midA")
                        te.matmul(out=y2p[0:64, :], lhsT=W2dup[:, 0:64],
                                  rhs=y1s[0][:, cs], start=True, stop=True)
                        te.matmul(out=y2p[64:128, :], lhsT=W2dup[:, 64:128],
                                  rhs=y1s[1][:, cs], start=True, stop=True)
                        sc.activation(out=y2s[:, cs], in_=y2p[:], func=AF.Relu,
                                      bias=cb2dup[:], scale=s2dup[:])
                    # y3
                    for cch in range(2):
                        cs = slice(cch * 512, (cch + 1) * 512)
                        y3p = psB.tile([128, 512], f32, tag="midA")
                        te.matmul(out=y3p[:], lhsT=cb3duprow[:],
                                  rhs=ones512row[:], start=True, stop=False)
                        te.matmul(out=y3p[0:64, :], lhsT=W3d2[0:64, 0:64],
                                  rhs=y2s[0:64, cs], start=False, stop=True)
                        te.matmul(out=y3p[64:128, :], lhsT=W3d2[64:128, 64:128],
                                  rhs=y2s[64:128, cs], start=False, stop=True,
                                  tile_position=(64, 64))
                        sc.activation(out=cats[0][64:128, cs], in_=y3p[0:64, :],
                                      func=AF.Relu, scale=s3col[:])
                        v.tensor_scalar(out=cats[1][0:64, cs],
                                        in0=y3p[64:128, :],
                                        scalar1=s3col[:], scalar2=0.0,
                                        op0=OP.mult, op1=OP.max)
                    # h1 per point
                    for half, n in enumerate(nn):
                        w4t = W4 if half == 0 else W4r
                        h1 = lsb.tile([128, 1024], bf16, tag=f"h1s{half}")
                        for cch in range(2):
                            cs = slice(cch * 512, (cch + 1) * 512)
                            h1p = psB.tile([128, 512], f32, tag="midB")
                            te.matmul(out=h1p[:], lhsT=cb4row[:],
                                      rhs=ones512row[:], start=True, stop=False)
                            te.matmul(out=h1p[:], lhsT=w4t[:],
                                      rhs=cats[half][:, cs],
                                      start=False, stop=True)
                            if cch == 0:
                                sc.activation(out=h1[:, cs], in_=h1p[:],
                                              func=AF.Relu, scale=s4col[:])
                            else:
                                v.tensor_scalar(out=h1[:, cs], in0=h1p[:],
                                                scalar1=s4col[:], scalar2=0.0,
                                                op0=OP.mult, op1=OP.max)
                        h1s.append(h1)
                    # h -> exp
                    texp = lsb.tile([128, 1024], bf16, tag="texp")
                    for cch in range(2):
                        cs = slice(cch * 512, (cch + 1) * 512)
                        hp = psB.tile([128, 512], f32, tag="midB")
                        te.matmul(out=hp[0:64, :], lhsT=W5dup[:, 0:64],
                                  rhs=h1s[1][:, cs], start=True, stop=True)
                        te.matmul(out=hp[64:128, :], lhsT=W5dup[:, 64:128],
                                  rhs=h1s[0][:, cs], start=True, stop=True)
                        sc.activation(out=texp[:, cs], in_=hp[:], func=AF.Exp,
                                      bias=cb5dup[:], scale=s5dup[:])
                    # num/den with fused relu: E = max(exp(z),1)
                    scr = lsb.tile([128, 1024], bf16, tag="scr")
                    v.scalar_tensor_tensor(
                        out=scr[64:128, :], in0=texp[64:128, :], scalar=1.0,
                        in1=cats[0][64:128, :], op0=OP.max, op1=OP.mult,
                        accum_out=numT[64:128, i:i + 1])
                    v.scalar_tensor_tensor(
                        out=scr[0:64, :], in0=texp[0:64, :], scalar=1.0,
                        in1=cats[1][0:64, :], op0=OP.max, op1=OP.mult,
                        accum_out=numT[0:64, i:i + 1])
                    scr2 = lsb.tile([128, 1024], bf16, tag="scr2")
                    v.tensor_scalar(
                        out=scr2[:], in0=texp[:], scalar1=1.0, scalar2=0.0,
                        op0=OP.max, op1=OP.add,
                        accum_out=denT[:, i:i + 1])

            # =========== pi_feat assembly + AllGather ===========
            with tc.tile_pool(name="ps2", bufs=2, space="PSUM") as ps2, \
                 tc.tile_pool(name="sb2", bufs=1) as sb2, \
                 tc.tile_pool(name="gsb", bufs=3) as gsb:
                denR = sb2.tile([128, 64], f32, tag="denR")
                v.reciprocal(out=denR[:], in_=denT[:])
                pfZ = sb2.tile([128, 64], f32, tag="pfZ")
                v.tensor_tensor(out=pfZ[:], in0=numT[:], in1=denR[:],
                                op=OP.mult)
                pfZ2 = sb2.tile([128, 64], f32, tag="pfZ2")
                v.tensor_copy(out=pfZ2[0:64, :], in_=pfZ[64:128, :])
                v.tensor_copy(out=pfZ2[64:128, :], in_=pfZ[0:64, :])
                pfp = ps2.tile([64, 128], f32, tag="tp")
                te.transpose(out=pfp[:], in_=pfZ2[:], identity=ident[:])
                pfs = sb2.tile([64, 128], f32, tag="pfs")
                v.tensor_copy(out=pfs[:], in_=pfp[:])
                nc.sync.dma_start(
                    out=pf_mine[:].rearrange("(i h) j -> i (h j)", h=2),
                    in_=pfs[:])
                if timing_mode:
                    nc.sync.dma_start(out=pf_full[0:M, :], in_=pf_mine[:])
                else:
                    nc.gpsimd.collective_compute(
                        "AllGather", OP.bypass,
                        replica_groups=[list(range(NC_))],
                        ins=[pf_mine[:]],
                        outs=[pf_full[:]],
                    )
                gxTt = sb2.tile([3, 2048], bf16, tag="gxTt")
                wbc = sb2.tile([3, 2048], bf16, tag="wbc")
                # gathers + transposes into channel-major
                for k in range(KNN):
                    gf = gsb.tile([128, 64], f32, tag="gf")
                    nc.gpsimd.indirect_dma_start(
                        out=gf[:], out_offset=None, in_=pf_full[:],
                        in_offset=bass.IndirectOffsetOnAxis(
                            ap=idxs[:, k:k + 1], axis=0))
                    gfp = ps2.tile([64, 128], f32, tag="tp")
                    te.transpose(out=gfp[:], in_=gf[:], identity=ident[:])
                    v.tensor_copy(out=gfT[:, k * 128:(k + 1) * 128], in_=gfp[:])
                    gx = gsb.tile([128, 4], f32, tag="gx")
                    nc.gpsimd.indirect_dma_start(
                        out=gx[:], out_offset=None, in_=P['wxyz4'][:],
                        in_offset=bass.IndirectOffsetOnAxis(
                            ap=idxs[:, k:k + 1], axis=0))
                    gxp = ps2.tile([4, 128], f32, tag="tp4")
                    te.transpose(out=gxp[:], in_=gx[:], identity=ident[:])
                    v.tensor_copy(out=gxTt[:, k * 128:(k + 1) * 128],
                                  in_=gxp[0:3, :])
                    v.tensor_copy(out=wbc[:, k * 128:(k + 1) * 128],
                                  in_=wxyzT_m[:])

                # pc10 channels: [wxyz, g_xyz, diff, euc]
                v.tensor_copy(out=pcrhs[0:3, :], in_=wbc[:])
                nc.sync.dma_start(out=pcrhs[3:6, :], in_=gxTt[:])
                dft = sb2.tile([3, 2048], bf16, tag="dft")
                v.tensor_tensor(out=dft[:], in0=gxTt[:], in1=wbc[:],
                                op=OP.subtract)
                nc.sync.dma_start(out=pcrhs[6:9, :], in_=dft[:])
                ones3b = sb2.tile([3, 1], bf16, tag="ones3b")
                v.memset(ones3b[:], 1.0)
                dsq = sb2.tile([3, 2048], bf16, tag="dsq")
                v.tensor_tensor(out=dsq[:], in0=dft[:], in1=dft[:],
                                op=OP.mult)
                euct = sb2.tile([1, 2048], bf16, tag="euct")
                for cch in range(4):
                    cs = slice(cch * 512, (cch + 1) * 512)
                    ecp = ps2.tile([1, 512], f32, tag="sm")
                    te.matmul(out=ecp[:], lhsT=ones3b[:], rhs=dsq[:, cs],
                              start=True, stop=True)
                    sc.activation(out=euct[:, cs], in_=ecp[:], func=AF.Sqrt)
                nc.sync.dma_start(out=pcrhs[9:10, :], in_=euct[:])
                # pc_enc
                rhs2a = sb2.tile([128, 2048], bf16, tag="rhs2a")
                for cch in range(4):
                    cs = slice(cch * 512, (cch + 1) * 512)
                    pcp = ps2.tile([64, 512], f32, tag="big")
                    te.matmul(out=pcp[:], lhsT=Wpc[:], rhs=pcrhs[:, cs],
                              start=True, stop=True)
                    sc.activation(out=rhs2a[0:64, cs], in_=pcp[:],
                                  func=AF.Relu, bias=cbpccol[:],
                                  scale=spccol[:])
                for k in range(KNN):
                    v.tensor_copy(out=rhs2a[64:128, k * 128:(k + 1) * 128],
                                  in_=wpT_m[:])
                # mlp2b layer 1
                h1bs = sb2.tile([128, 2048], bf16, tag="h1bs")
                for cch in range(4):
                    cs = slice(cch * 512, (cch + 1) * 512)
                    h1bp = ps2.tile([128, 512], f32, tag="big")
                    te.matmul(out=h1bp[:], lhsT=W6a[:], rhs=rhs2a[:, cs],
                              start=True, stop=False)
                    te.matmul(out=h1bp[:], lhsT=W6b[:], rhs=gfT[:, cs],
                              start=False, stop=True)
                    sc.activation(out=h1bs[:, cs], in_=h1bp[:], func=AF.Relu,
                                  bias=cb6col[:], scale=s6col[:])
                # mlp2b layer 2 -> exp
                texp2 = sb2.tile([64, 2048], bf16, tag="texp2")
                for cch in range(4):
                    cs = slice(cch * 512, (cch + 1) * 512)
                    z2p = ps2.tile([64, 512], f32, tag="big")
                    te.matmul(out=z2p[:], lhsT=W7[:], rhs=h1bs[:, cs],
                              start=True, stop=True)
                    sc.activation(out=texp2[:, cs], in_=z2p[:], func=AF.Exp,
                                  bias=cb7col[:], scale=s7col[:])
                # num2 / den2 with segment reduce over k (free stride 128)
                scr2a = sb2.tile([64, 2048], bf16, tag="scr2a")
                v.scalar_tensor_tensor(out=scr2a[:], in0=texp2[:], scalar=1.0,
                                       in1=gfT[:], op0=OP.max, op1=OP.mult)
                num2 = sb2.tile([64, 128], f32, tag="num2")
                v.tensor_reduce(
                    out=num2[:],
                    in_=scr2a[:].rearrange("p (k n) -> p n k", k=KNN),
                    axis=mybir.AxisListType.X, op=OP.add)
                scr2b = sb2.tile([64, 2048], bf16, tag="scr2b")
                v.tensor_scalar(out=scr2b[:], in0=texp2[:], scalar1=1.0,
                                scalar2=None, op0=OP.max)
                den2 = sb2.tile([64, 128], f32, tag="den2")
                v.tensor_reduce(
                    out=den2[:],
                    in_=scr2b[:].rearrange("p (k n) -> p n k", k=KNN),
                    axis=mybir.AxisListType.X, op=OP.add)
                den2r = sb2.tile([64, 128], f32, tag="den2r")
                v.reciprocal(out=den2r[:], in_=den2[:])
                outT = sb2.tile([64, 128], f32, tag="outT")
                v.tensor_tensor(out=outT[:], in0=num2[:], in1=den2r[:],
                                op=OP.mult)
                outp = ps2.tile([128, 64], f32, tag="tp")
                te.transpose(out=outp[:], in_=outT[:],
                             identity=ident[0:64, 0:64])
                outs = sb2.tile([128, 64], f32, tag="outs")
                v.tensor_copy(out=outs[:], in_=outp[:])
                nc.sync.dma_start(out=out_p[:], in_=outs[:])

    nc.finalize()
    return nc


_NC_CACHE = None


def kernel(**inputs):
    global LAST_EXEC_NS, LAST_RESULT, _NC_CACHE
    in_maps = _host_prep(inputs)
    if _NC_CACHE is None:
        _NC_CACHE = _build()
    import os
    res = run_bass_kernel_spmd(
        _NC_CACHE, in_maps, core_ids=list(range(NC_)),
        trace=bool(os.environ.get("KERNEL_TRACE")),
    )
    LAST_RESULT = res
    LAST_EXEC_NS = res.exec_time_ns
    out = np.concatenate([res.results[i]["out"] for i in range(NC_)], 0)
    return out[None].astype(np.float32)
